# revision 1
# baseline (speedup 1.0000x reference)
"""Trainium2 Bass kernel for nn_Encoder_23124103922122 (segment_reduce).

Math (per rank r of 6, labels lab_r[0..4095] in [0,256)):
    seg_r[b, g]  = sum_{i: lab_r[i]==g} F[b, i]          (segment sum)
    out[b, j, r] = seg_r[b, lab_r[j]]                     (gather back)
    out[b, j, 6] = F[b, j]                                (identity channel)

Implementation: both stages as one-hot matmuls on TensorE.
    stage 1: psum_seg[b, g] += F_T[i_tile].T @ M[i_tile, g]      (M one-hot of labels)
    stage 2: psum_out[b, j] = seg_T[g, b].T @ M_T[g, j]          (M_T one-hot, g on partitions)
One-hot matrices built on DVE via tensor_scalar(is_equal) from iota/label tables
that the host passes in as extra inputs (a few per rank on ACT via
relu(1-|iota-lab|), exact for integers). Matmul operands use float32r
(~13-bit-mantissa fp32) for 4x PE throughput vs plain fp32; transposes stay
plain fp32 (f32r transpose-mode is broken on HW). The [B, N, 7] channel
interleave is done by strided ACT copies PSUM->SBUF; output streams out in
j-chunks so the 14.6MB/core store overlaps stage-2 compute.

Sharding: data-parallel over batch B=1024 -> 8 cores x 128 rows. Labels & tables
replicated. No cross-device communication.

Cost-model timeline (per core): ~91us, roughly at the output-bandwidth
roofline (stage 1 is DVE/ACT mask-build bound ~40us; stage 2 is HBM-write
bound). Measured rel err vs fp32 reference: 1.5e-4.

Note: walrus in this container accepts at most ONE sync-wait per instruction
(two on EventSemaphore); _legalize_waits() post-processes the Tile-scheduled
program to satisfy that (drop provably-redundant same-engine waits, hoist the
rest onto EventSemaphore carriers).
"""

import sys

if "/opt/trn_rl_repo" not in sys.path:
    sys.path.insert(0, "/opt/trn_rl_repo")

from contextlib import ExitStack

import ml_dtypes
import numpy as np

import concourse.bass as bass
import concourse.mybir as mybir
import concourse.tile as tile
from concourse.bass import ts
from concourse.bass_utils import run_bass_kernel_spmd

B, N, R, G = 1024, 4096, 6, 256
NCORES = 8
BL = B // NCORES  # 128 batch rows per core
P = 128
NT = N // P  # 32 genus tiles
JC = 512  # stage-2 j-chunk width
NJ = N // JC
F32 = mybir.dt.float32
F32R = mybir.dt.float32r
BF16 = mybir.dt.bfloat16


def _r(ap):
    """View an fp32 AP as float32r for 4x-rate PE consumption."""
    return ap.bitcast(F32R) if ap.dtype == F32 else ap

# Compute dtype for the matmul operands (one-hots, F_T, seg_T).
# f32 = exact; bf16 = ~2x faster DVE/PE but ~3e-3 relative error.
COMPUTE_DT = F32
CD_NP = np.float32 if COMPUTE_DT == F32 else ml_dtypes.bfloat16

_cache: dict = {}

# Engine -> prefix of the semaphore names its compute instructions increment.
_ENGINE_SEM_PREFIX = {
    mybir.EngineType.PE: "PE",
    mybir.EngineType.DVE: "DVE",
    mybir.EngineType.Activation: "Activation",
    mybir.EngineType.Pool: "Pool",
    mybir.EngineType.SP: "SP",
}


def _legalize_waits(nc):
    """Walrus only accepts 1 sync-wait per instruction (2 on EventSemaphore),
    but the Tile scheduler can emit more. Post-pass:
      1. drop waits on the instruction's own engine semaphore that are already
         satisfied by same-engine program order (compute completion is in-order
         and sem targets are absolute), and
      2. hoist remaining excess waits onto EventSemaphore carrier instructions
         inserted just before the instruction on the same engine.
    """
    ev_id = 0
    for f in nc.m.functions:
        for blk in f.blocks:
            insts = blk.instructions
            sem_incs: dict = {}  # (engine, sem_name) -> cumulative inc in stream
            new_insts = []
            for inst in insts:
                si = inst.sync_info
                if si is not None and si.on_wait:
                    cap = 2 if isinstance(inst, mybir.InstEventSemaphore) else 1
                    eng = inst.engine
                    pfx = _ENGINE_SEM_PREFIX.get(eng)
                    kept = []
                    for w in si.on_wait:
                        sem_eng = w.ant_name.rsplit("_", 1)[0]
                        if (
                            pfx is not None
                            and sem_eng == pfx
                            and w.wait_mode == "sem-ge-imm"
                            and sem_incs.get((eng, w.ant_name), 0) >= w.wait_value
                        ):
                            continue  # satisfied by same-engine execution order
                        kept.append(w)
                    while len(kept) > cap:
                        ncarry = min(2, len(kept) - cap + 1)
                        carry, kept = kept[:ncarry], kept[ncarry:]
                        ev = mybir.InstEventSemaphore(
                            name=f"EVW-{ev_id}", ins=[], outs=[]
                        )
                        ev_id += 1
                        ev.engine = eng
                        ev.sync_info = mybir.SyncInfo(on_wait=carry, on_update=[])
                        new_insts.append(ev)
                    inst.sync_info = mybir.SyncInfo(
                        on_wait=kept, on_update=si.on_update
                    )
                si = inst.sync_info
                if si is not None:
                    for u in si.on_update:
                        if u.update_mode == "sem-inc":
                            key = (inst.engine, u.ant_name)
                            sem_incs[key] = sem_incs.get(key, 0) + u.update_value
                new_insts.append(inst)
            if len(new_insts) != len(insts):
                insts[:] = new_insts


def _build_nc():
    nc = bass.Bass("TRN2", debug=False, num_devices=NCORES)

    f_in = nc.dram_tensor("f_in", [BL, N], F32, kind="ExternalInput").ap()
    # tabs_f32[p, 0:2] = iota_p (p + 128*k); tabs_f32[p, 2 + r*NT + t] = labels[r, t*128+p]
    # (per-partition scalar operands for is_equal -- must be f32)
    tabs_f32 = nc.dram_tensor(
        "tabs_f32", [P, 2 + R * NT], F32, kind="ExternalInput"
    ).ap()
    # tabs_cd[p, 0:G] = iota_g (col index); tabs_cd[p, G:G+P] = identity
    tabs_cd = nc.dram_tensor(
        "tabs_cd", [P, G + P], COMPUTE_DT, kind="ExternalInput"
    ).ap()
    # lab_bf[r, j] = labels[r, j] (bf16, partition-broadcast source for stage 2)
    lab_bf = nc.dram_tensor("lab_bf", [R, N], BF16, kind="ExternalInput").ap()
    out = nc.dram_tensor("out", [BL, N, R + 1], F32, kind="ExternalOutput").ap()

    with ExitStack() as ctx:
        tc = ctx.enter_context(tile.TileContext(nc))

        const = ctx.enter_context(tc.tile_pool(name="const", bufs=1))
        fpool = ctx.enter_context(tc.tile_pool(name="fpool", bufs=1))
        mpool = ctx.enter_context(tc.tile_pool(name="mpool", bufs=24))
        segp = ctx.enter_context(tc.tile_pool(name="segp", bufs=1))
        mt2p = ctx.enter_context(tc.tile_pool(name="mt2p", bufs=8))
        outp = ctx.enter_context(tc.tile_pool(name="outp", bufs=3))
        ps_tr = ctx.enter_context(tc.tile_pool(name="ps_tr", bufs=2, space="PSUM"))

        # ---- constants + F load. Order matters: the tiny tables go first so
        # DVE mask-building starts ~1us in; then F (transposes); the big 6MB
        # lab_bc broadcast is gated behind the F load via a Pool-engine dep so
        # it streams during stage-1 compute instead of starving startup DMA. ----
        tf32_sb = const.tile([P, 2 + R * NT], F32)
        nc.sync.dma_start(tf32_sb[:], tabs_f32)
        tcd_sb = const.tile([P, G + P], COMPUTE_DT)
        nc.sync.dma_start(tcd_sb[:], tabs_cd)
        f_sb = fpool.tile([P, N], F32)
        f_dmas = [
            nc.sync.dma_start(
                f_sb[:, q * (N // 4) : (q + 1) * (N // 4)],
                f_in[:, q * (N // 4) : (q + 1) * (N // 4)],
            )
            for q in range(4)
        ]
        # lab_bc[p, r, j] = labels[r, j] for every partition p.
        # Explicit dep: the 6MB broadcast DMA must start only after the F load
        # has finished -- otherwise it hogs the DMA engines while everything
        # else waits on F/tables.
        lab_bc = const.tile([P, R, N], BF16)
        lab_dma = nc.gpsimd.dma_start(lab_bc[:], lab_bf.partition_broadcast(P))
        from concourse.tile import add_dep_helper

        add_dep_helper(
            lab_dma.ins, f_dmas[-1].ins, reason="delay lab_bc after F load"
        )
        iota_p_sb = tf32_sb[:, 0:2]
        labT_sb = tf32_sb[:, 2:]
        iota_g_sb = tcd_sb[:, 0:G]
        ident_sb = tcd_sb[:, G:]

        # Prewarm: absorb each const-DMA semaphore into the DVE/PE vector
        # clocks with one cheap op, so the TensorScalarPtr ops in the hot
        # loops never carry more than one sync wait (HW limit is 1 there).
        warm = const.tile([P, 4], COMPUTE_DT)
        nc.vector.tensor_copy(warm[:, 0:1], tf32_sb[:, 0:1])
        nc.vector.tensor_copy(warm[:, 1:2], tcd_sb[:, 0:1])
        with tc.tile_pool(name="ps_warm", bufs=1, space="PSUM") as ps_warm:
            wps = ps_warm.tile([P, P], COMPUTE_DT)
            nc.tensor.transpose(wps[:], ident_sb[:], ident_sb[:])
            nc.scalar.copy(warm[:, 3:4], wps[:, 0:1])
        f_cd = f_sb

        f_t = fpool.tile([P, N], F32R)  # col t*128.. holds transpose of tile t
        for t in range(NT):
            ps = ps_tr.tile([P, P], COMPUTE_DT, tag="tr")
            nc.tensor.transpose(ps[:], f_cd[:, ts(t, P)], ident_sb[:])
            nc.scalar.copy(f_t[:, ts(t, P)], ps[:])

        # ---- stage 1: seg[b, g] per rank, accumulated over the 32 genus tiles.
        # Rank-major so each rank's seg transposes overlap the next rank's
        # matmuls. Most one-hot masks are built on DVE (is_equal); every 6th
        # goes to the otherwise-idle ACT engine as relu(1 - |iota - lab|)
        # (exact for integer-valued inputs). ----
        seg_t = []
        m2_pre = {}
        with tc.tile_pool(name="ps_seg", bufs=1, space="PSUM") as ps_seg:
            seg_psum = [
                ps_seg.tile([P, G], F32, tag=f"seg{r}", name=f"seg_ps{r}")
                for r in range(R)
            ]
            for r in range(R):
                for t in range(NT):
                    col = r * NT + t
                    mt = mpool.tile([P, G], F32R, tag="m1")
                    if r >= 1 and t % 8 == 7:
                        tabs_ = mpool.tile([P, G], F32, tag="mabs")
                        nc.scalar.activation(
                            tabs_[:],
                            iota_g_sb[:],
                            mybir.ActivationFunctionType.Abs,
                            bias=labT_sb[:, col : col + 1],
                            scale=-1.0,
                        )
                        nc.scalar.activation(
                            mt[:],
                            tabs_[:],
                            mybir.ActivationFunctionType.Relu,
                            bias=1.0,
                            scale=-1.0,
                        )
                    else:
                        nc.vector.tensor_scalar(
                            mt[:],
                            iota_g_sb[:],
                            labT_sb[:, col : col + 1],
                            None,
                            op0=mybir.AluOpType.is_equal,
                        )
                    nc.tensor.matmul(
                        seg_psum[r][:],
                        f_t[:, ts(t, P)],
                        mt[:],
                        start=(t == 0),
                        stop=(t == NT - 1),
                    )

                # ---- seg -> seg_T (g on partitions) for this rank ----
                s_sb = segp.tile([P, G], COMPUTE_DT, tag=f"segsb{r}", name=f"ssb{r}")
                nc.scalar.copy(s_sb[:], seg_psum[r][:])
                st = segp.tile([P, G], F32R, tag=f"segT{r}", name=f"st{r}")
                for g in range(2):
                    ps = ps_tr.tile([P, P], COMPUTE_DT, tag="tr")
                    nc.tensor.transpose(ps[:], s_sb[:, ts(g, P)], ident_sb[:])
                    nc.scalar.copy(st[:, ts(g, P)], ps[:])
                seg_t.append(st)

        # ---- stage 2: out[b, j] = seg[b, lab[j]] per rank, interleave, store ----
        # absorb the lab_bc broadcast-DMA semaphore now (DVE was busy with
        # stage-1 masks while it streamed in)
        nc.vector.tensor_copy(warm[:, 2:3], lab_bc[:, 0, 0:1])
        # small chunks at the start (first out-DMA fires sooner) and at the
        # end (short final drain); big chunks in the middle for DMA efficiency
        widths = [512] * 7 + [256, 256]
        assert sum(widths) == N
        with tc.tile_pool(name="ps_o", bufs=4, space="PSUM") as ps_o:
            j0 = 0
            for c, w in enumerate(widths):
                o_sb = outp.tile([P, w, R + 1], F32, tag="osb", name=f"osb{c}")
                for r in range(R):
                    po = ps_o.tile([P, w], F32, tag="po", name=f"po{c}_{r}")
                    for g in range(2):
                        m2 = mt2p.tile([P, w], F32R, tag="m2", name=f"m2_{c}_{r}_{g}")
                        nc.vector.tensor_scalar(
                            m2[:],
                            lab_bc[:, r, j0 : j0 + w],
                            iota_p_sb[:, g : g + 1],
                            None,
                            op0=mybir.AluOpType.is_equal,
                        )
                        nc.tensor.matmul(
                            po[:],
                            seg_t[r][:, ts(g, P)],
                            m2[:],
                            start=(g == 0),
                            stop=(g == 1),
                        )
                    nc.scalar.copy(o_sb[:, :, r], po[:])
                nc.scalar.copy(o_sb[:, :, R], f_sb[:, j0 : j0 + w])
                nc.sync.dma_start(out[:, j0 : j0 + w, :], o_sb[:])
                j0 += w

    _legalize_waits(nc)
    return nc


def _host_tables():
    """tabs_cd aux table shared by all cores (cached)."""
    if "tabs_cd" not in _cache:
        iota_g = np.tile(np.arange(G, dtype=np.float64), (P, 1))
        ident = np.eye(P, dtype=np.float64)
        _cache["tabs_cd"] = np.concatenate([iota_g, ident], axis=1).astype(CD_NP)
    return _cache["tabs_cd"]


def kernel(F_genus: np.ndarray, labels: np.ndarray) -> np.ndarray:
    F_genus = np.ascontiguousarray(F_genus, dtype=np.float32)
    labels = np.ascontiguousarray(labels, dtype=np.int32)
    assert F_genus.shape == (B, N) and labels.shape == (R, N)

    tabs_cd = _host_tables()
    # labT[p, r*NT + t] = labels[r, t*128 + p]
    labT = np.transpose(labels.reshape(R, NT, P), (2, 0, 1)).reshape(P, R * NT)
    iota_p = np.arange(P, dtype=np.float64)[:, None] + 128.0 * np.arange(2)[None, :]
    tabs_f32 = np.concatenate([iota_p, labT], axis=1).astype(np.float32)
    lab_bf = labels.astype(ml_dtypes.bfloat16)

    in_maps = []
    for c in range(NCORES):
        in_maps.append(
            {
                "f_in": F_genus[c * BL : (c + 1) * BL],
                "tabs_f32": tabs_f32,
                "tabs_cd": tabs_cd,
                "lab_bf": lab_bf,
            }
        )

    # The first execution of a freshly compiled NEFF occasionally hits a
    # transient NRT_EXEC_UNIT_UNRECOVERABLE; a rebuild + retry recovers.
    last_err = None
    for attempt in range(3):
        try:
            if "nc" not in _cache:
                _cache["nc"] = _build_nc()
            res = run_bass_kernel_spmd(
                _cache["nc"], in_maps, core_ids=list(range(NCORES))
            )
            return np.concatenate([r["out"] for r in res.results], axis=0)
        except Exception as e:  # noqa: BLE001
            last_err = e
            _cache.pop("nc", None)
            import time as _time

            _time.sleep(3.0)
    raise last_err



# revision 31
# speedup vs baseline: 1.1734x; 1.1734x over previous
"""Trainium2 Bass kernel for nn_Encoder_23124103922122 (segment_reduce).

Math (per rank r of 6, labels lab_r[0..4095] in [0,256)):
    seg_r[b, g]  = sum_{i: lab_r[i]==g} F[b, i]          (segment sum)
    out[b, j, r] = seg_r[b, lab_r[j]]                     (gather back)
    out[b, j, 6] = F[b, j]                                (identity channel)

v2 design (cost-model-driven; DMA is the wall at ~47us/core):
  - Host pre-transposes F: ft[i, t*128+b] = F[b, t*128+i] (fp16) and sends
    F b-major as fp16 too -- no on-chip F transposes at all.
  - Everything on PE is pure fp16 (masks are exact one-hots; F/seg rounding
    ~5e-4): moving-operand fp16 = full PE rate.
  - Stage 1: one-hot masks [i,g] built per (rank, i-tile) on DVE (4x fp16
    mode) with a minority on Pool/ACT; 32 accumulating matmuls per rank.
  - Stage 2: label row broadcast to all partitions via DVE stream_shuffle on
    u32-bitcast quarters (no 17.5us DMA broadcast, no Pool broadcast); m2
    one-hot [g, j] built on DVE; per 64-j chunk, 6 rank matmuls + 1
    identity matmul write channel-major into ONE [128, 7, 64] PSUM bank;
    a single strided copy (ACT/DVE/Pool rotated) interleaves PSUM->SBUF
    [128, 64, 7]; stores stream from SBUF on the SP + ACT DMA queues.
  - PE p-state warmup chain of tiny matmuls before stage 1.

Sharding: data-parallel over batch B=1024 -> 8 cores x 128 rows. Labels &
tables replicated. No cross-device communication.

Note: walrus in this container accepts at most ONE sync-wait per instruction
(two on EventSemaphore); _legalize_waits() post-processes the Tile-scheduled
program to satisfy that.
"""

import sys

if "/opt/trn_rl_repo" not in sys.path:
    sys.path.insert(0, "/opt/trn_rl_repo")

from contextlib import ExitStack

import numpy as np

import concourse.bass as bass
import concourse.mybir as mybir
import concourse.tile as tile
from concourse.bass import ts
from concourse.bass_utils import run_bass_kernel_spmd

B, N, R, G = 1024, 4096, 6, 256
NCORES = 8
BL = B // NCORES  # 128 batch rows per core
P = 128
NT = N // P  # 32 genus tiles
JC = 64  # stage-2 chunk width (64 j x 7 ch = 448 f32 in one PSUM bank)
NCH = N // JC  # 64 chunks
QW = 1024  # m2/lab_bc quarter width in j
NQ = N // QW  # 4
CQ = QW // JC  # 16 chunks per quarter
F32 = mybir.dt.float32
FP16 = mybir.dt.float16
U32 = mybir.dt.uint32
TAB = G + P + 2 * (2 + R * NT + P)  # 1028 fp16 cols of packed tables

_cache: dict = {}

# Engine -> prefix of the semaphore names its compute instructions increment.
_ENGINE_SEM_PREFIX = {
    mybir.EngineType.PE: "PE",
    mybir.EngineType.DVE: "DVE",
    mybir.EngineType.Activation: "Activation",
    mybir.EngineType.Pool: "Pool",
    mybir.EngineType.SP: "SP",
}


def _legalize_waits(nc):
    """Walrus only accepts 1 sync-wait per instruction (2 on EventSemaphore),
    but the Tile scheduler can emit more. Post-pass:
      1. drop waits on the instruction's own engine semaphore that are already
         satisfied by same-engine program order (compute completion is in-order
         and sem targets are absolute), and
      2. hoist remaining excess waits onto EventSemaphore carrier instructions
         inserted just before the instruction on the same engine.
    """
    ev_id = 0
    for f in nc.m.functions:
        for blk in f.blocks:
            insts = blk.instructions
            sem_incs: dict = {}  # (engine, sem_name) -> cumulative inc in stream
            new_insts = []
            for inst in insts:
                si = inst.sync_info
                if si is not None and si.on_wait:
                    cap = 2 if isinstance(inst, mybir.InstEventSemaphore) else 1
                    eng = inst.engine
                    pfx = _ENGINE_SEM_PREFIX.get(eng)
                    kept = []
                    for w in si.on_wait:
                        sem_eng = w.ant_name.rsplit("_", 1)[0]
                        if (
                            pfx is not None
                            and sem_eng == pfx
                            and w.wait_mode == "sem-ge-imm"
                            and sem_incs.get((eng, w.ant_name), 0) >= w.wait_value
                        ):
                            continue  # satisfied by same-engine execution order
                        kept.append(w)
                    while len(kept) > cap:
                        ncarry = min(2, len(kept) - cap + 1)
                        carry, kept = kept[:ncarry], kept[ncarry:]
                        ev = mybir.InstEventSemaphore(
                            name=f"EVW-{ev_id}", ins=[], outs=[]
                        )
                        ev_id += 1
                        ev.engine = eng
                        ev.sync_info = mybir.SyncInfo(on_wait=carry, on_update=[])
                        new_insts.append(ev)
                    inst.sync_info = mybir.SyncInfo(
                        on_wait=kept, on_update=si.on_update
                    )
                si = inst.sync_info
                if si is not None:
                    for u in si.on_update:
                        if u.update_mode == "sem-inc":
                            key = (inst.engine, u.ant_name)
                            sem_incs[key] = sem_incs.get(key, 0) + u.update_value
                new_insts.append(inst)
            if len(new_insts) != len(insts):
                insts[:] = new_insts


# Per-rank stage-1 mask engine split (32 tiles): DVE carries most, Pool and
# ACT absorb the rest so DVE can also build the stage-2 masks in time.
_S1_POOL_T = {2, 6, 11, 16, 21, 26, 30}  # 7 per rank on Pool
_S1_ACT_T = {4, 14, 24}  # 3 per rank on ACT

# Stage-2 chunk-copy engine rotation (PSUM->SBUF interleave): per 8 chunks.
_COPY_ROT = ["act", "dve", "pool", "act", "dve", "act", "dve", "pool"]


def _build_nc():
    nc = bass.Bass("TRN2", debug=False, num_devices=NCORES)

    # ft_in layout: [tables header (TAB fp16 cols) | ft columns (N)]
    #   hdr[0:G] = iota_g fp16; hdr[G:G+P] = identity fp16;
    #   hdr[G+P : G+P+644] = f32 table bitcast as fp16 pairs:
    #       f32[0:2] iota_p, f32[2:194] labT, f32[194:322] identity f32
    ft_in = nc.dram_tensor("ft_in", [P, TAB + N], FP16, kind="ExternalInput").ap()
    fb_in = nc.dram_tensor("fb_in", [P, N], FP16, kind="ExternalInput").ap()
    lab4_in = nc.dram_tensor("lab4_in", [R, N], FP16, kind="ExternalInput").ap()
    out = nc.dram_tensor("out", [BL, N, R + 1], F32, kind="ExternalOutput").ap()
    # tiny sink for the PE-warmup results (BIR verifier needs every location read)
    wsink = nc.dram_tensor("wsink", [P, 8], F32, kind="ExternalOutput").ap()

    with ExitStack() as ctx:
        tc = ctx.enter_context(tile.TileContext(nc))

        const = ctx.enter_context(tc.tile_pool(name="const", bufs=1))
        fpool = ctx.enter_context(tc.tile_pool(name="fpool", bufs=1))
        lbc = ctx.enter_context(tc.tile_pool(name="lbc", bufs=6))
        m1p = ctx.enter_context(tc.tile_pool(name="m1p", bufs=32))
        mabsp = ctx.enter_context(tc.tile_pool(name="mabsp", bufs=6))
        m2p = ctx.enter_context(tc.tile_pool(name="m2p", bufs=18))
        segp = ctx.enter_context(tc.tile_pool(name="segp", bufs=1))
        outp = ctx.enter_context(tc.tile_pool(name="outp", bufs=4))

        # ---- constant tables first (tiny), then F streams ----
        from concourse.tile import add_dep_helper

        # one combined header+ft tensor: tables arrive with the very first
        # DMA; lab4 goes last (not needed until mid-stage-1)
        hdr = fpool.tile([P, TAB + N], FP16)
        HCUTS = [0, TAB + 512, TAB + 2048, TAB + N]
        d_ft = []
        for ci in range(len(HCUTS) - 1):
            a, b = HCUTS[ci], HCUTS[ci + 1]
            d_ft.append(nc.sync.dma_start(hdr[:, a:b], ft_in[:, a:b]))
            if ci:
                add_dep_helper(d_ft[ci].ins, d_ft[ci - 1].ins, reason="queue order")
        fb = fpool.tile([P, N], FP16)
        d_fb = nc.sync.dma_start(fb[:], fb_in)
        add_dep_helper(d_fb.ins, d_ft[-1].ins, reason="fb after ft")
        lab4 = const.tile([P, N], FP16)
        d_lab = [
            nc.sync.dma_start(lab4[32 * a : 32 * a + R, :], lab4_in)
            for a in range(4)
        ]
        add_dep_helper(d_lab[0].ins, d_fb.ins, reason="lab4 last")

        ft = hdr[:, TAB : TAB + N]
        f32v = hdr[:, G + P : TAB].bitcast(F32)  # [P, 322] f32 view
        iota_p = f32v[:, 0:2]
        labT = f32v[:, 2 : 2 + R * NT]
        ident_f = f32v[:, 2 + R * NT : 322]
        iota_g = hdr[:, 0:G]
        ident_h = hdr[:, G : G + P]

        # Prewarm: absorb const-DMA semaphores into engine clocks with cheap
        # ops so hot-loop instructions carry at most one sync wait each.
        wsrc = const.tile([P, P], FP16)
        nc.vector.memset(wsrc[:], 0)
        warm = const.tile([P, 8], F32)
        nc.vector.memset(warm[:], 0)
        # consume the preamble const tiles (BIR verifier wants readers)
        for _d, _v in (
            (mybir.dt.float32, 0.0),
            (mybir.dt.float32, 1.0),
            (mybir.dt.bfloat16, 1.0),
            (mybir.dt.uint8, 127),
        ):
            nc.vector.tensor_copy(warm[:, 4:5], nc.const_aps.aps[(_d, _v)])

        st = []  # seg_T fp16 per rank: st[r][g_loc, 128*gt + b]
        m2 = {}  # (r, gt, pi) -> [P, 1024] fp16 tile (first w cols valid)
        lab_bc = {}  # (r, pi) -> [P, 1024] fp16 broadcast label row

        # j-space piecing for lab_bc/m2: a small piece 0 (cheap to build on
        # DVE before the first store chunk), then 1024-wide pieces built
        # during the store phase.
        PIECES = [256] * 4 + [512] * 6  # sums to N
        POFF = [0]
        for w in PIECES:
            POFF.append(POFF[-1] + w)
        # store chunk widths (graduated: small first for early store start)
        CHUNKS = [64, 64, 128] + [256] * 14 + [128, 64, 64]  # sums to N
        COFF = [0]
        for w in CHUNKS:
            COFF.append(COFF[-1] + w)

        def piece_of(j0, j1):
            for pi in range(len(PIECES)):
                if POFF[pi] <= j0 and j1 <= POFF[pi + 1]:
                    return pi
            raise AssertionError((j0, j1))

        def shuffle_op(r, pi):
            w = PIECES[pi]
            t_l = lbc.tile([P, 1024], FP16, tag=f"lbc{pi % 2}", name=f"lbc{r}_{pi}")
            nc.vector.stream_shuffle(
                t_l[:, 0:w].bitcast(U32),
                lab4[:, POFF[pi] : POFF[pi] + w].bitcast(U32),
                [r] * 32,
            )
            lab_bc[(r, pi)] = t_l

        def m2_op(r, gt, pi):
            w = PIECES[pi]
            t_m = m2p.tile([P, 1024], FP16, tag=f"m2_{pi % 2}", name=f"m2_{r}_{gt}_{pi}")
            nc.vector.tensor_scalar(
                t_m[:, 0:w],
                lab_bc[(r, pi)][:, 0:w],
                iota_p[:, gt : gt + 1],
                None,
                op0=mybir.AluOpType.is_equal,
            )
            m2[(r, gt, pi)] = t_m

        with tc.tile_pool(name="ps_seg", bufs=1, space="PSUM") as ps_seg, \
                tc.tile_pool(name="ps_tr", bufs=2, space="PSUM") as ps_tr:
            # PE p-state warmup: small dependency-free matmuls start the ramp
            # clock before the real stage-1 work arrives.
            wps = [
                ps_tr.tile([P, G], F32, tag="tr", name=f"w{i}") for i in range(2)
            ]
            for i in range(14):
                nc.tensor.matmul(
                    wps[i % 2][:, 0:64],
                    wsrc[:],
                    wsrc[:, 0:64],
                    start=True,
                    stop=True,
                    skip_group_check=True,
                )
            # absorb the warm chain (verifier needs readers)
            nc.scalar.copy(warm[:, 6:7], wps[0][:, 0:1])
            nc.scalar.copy(warm[:, 7:8], wps[1][:, 0:1])
            nc.scalar.dma_start(wsink, warm[:])

            seg_tiles = [
                ps_seg.tile([P, G], F32, tag=f"segp{r}", name=f"segp{r}")
                for r in range(R)
            ]
            seg_ps = [seg_tiles[r][:] for r in range(R)]

            def s1_mask(r, t):
                col = r * NT + t
                idx = t * R + r
                m1 = m1p.tile([P, G], FP16, tag="m1")
                if idx % 10 == 3:  # ACT via abs+relu (exact for ints)
                    tmp = mabsp.tile([P, G], F32, tag="mabs")
                    nc.scalar.activation(
                        tmp[:],
                        iota_g,
                        mybir.ActivationFunctionType.Abs,
                        bias=labT[:, col : col + 1],
                        scale=-1.0,
                    )
                    nc.scalar.activation(
                        m1[:],
                        tmp[:],
                        mybir.ActivationFunctionType.Relu,
                        bias=1.0,
                        scale=-1.0,
                    )
                elif idx % 9 in (1, 5):  # Pool
                    nc.gpsimd.tensor_scalar(
                        m1[:],
                        iota_g,
                        labT[:, col : col + 1],
                        None,
                        op0=mybir.AluOpType.is_equal,
                    )
                else:  # DVE (4x fp16 mode)
                    nc.vector.tensor_scalar(
                        m1[:],
                        iota_g,
                        labT[:, col : col + 1],
                        None,
                        op0=mybir.AluOpType.is_equal,
                    )
                return m1

            def s1_matmul(r, t, m1):
                nc.tensor.matmul(
                    seg_ps[r],
                    ft[:, ts(t, P)],
                    m1[:],
                    start=(t == 0),
                    stop=(t == NT - 1),
                )

            # DVE inserts during stage 1: piece-0 shuffles + m2 (cheap) and
            # piece-1 shuffles, spread across the tile loop.
            inserts = (
                [("sh", r, 0) for r in range(R)]
                + [("m2", r, gt, 0) for r in range(R) for gt in range(2)]
                + [("sh", r, 1) for r in range(R)]
                + [("m2", r, gt, 1) for r in range(R) for gt in range(2)]
            )
            # ~1.4 ops per tile from tile 2 on
            ins_at = {}
            for k, op in enumerate(inserts):
                ins_at.setdefault(2 + (2 * k) // 3, []).append(op)

            # tile-major over t=0..27 (tile t only needs its own ft quarter,
            # so PE never stalls on the F load), then per-rank finish for
            # t=28..31 so each rank's seg->seg_T chain overlaps the next
            # rank's matmuls.
            for t in range(NT - 4):
                for r in range(R):
                    s1_matmul(r, t, s1_mask(r, t))
                for op in ins_at.get(t, ()):
                    if op[0] == "sh":
                        shuffle_op(op[1], op[2])
                    else:
                        m2_op(op[1], op[2], op[3])
            # finish all remaining matmuls rank-by-rank, chains as each
            # rank completes; the copies ride the idle Pool engine so ACT is
            # free for the first chunk copies.
            pts = []
            for r in range(R):
                for t in range(NT - 4, NT):
                    s1_matmul(r, t, s1_mask(r, t))
                s_sb = segp.tile([P, G], F32, tag=f"ssb{r}", name=f"ssb{r}")
                nc.scalar.copy(s_sb[:], seg_ps[r])
                pt = ps_tr.tile([P, G], F32, tag="tr")
                for gt in range(2):
                    nc.tensor.transpose(
                        pt[:, ts(gt, P)], s_sb[:, ts(gt, P)], ident_f
                    )
                pts.append(pt)
                st_r = segp.tile([P, G], FP16, tag=f"st{r}", name=f"st{r}")
                nc.scalar.copy(st_r[:], pt[:])
                st.append(st_r)

        # ---- stage 2: per chunk, channel-major PSUM, one interleave copy
        # (ACT/Pool alternating), store on the SP queue. DVE runs the
        # remaining piece builds as one serial stream; it stays ahead of the
        # 2.55us/chunk store rate. ----
        dve_rest = []
        for pi in range(2, len(PIECES)):
            dve_rest += [("sh", r, pi) for r in range(R)]
            dve_rest += [("m2", r, gt, pi) for r in range(R) for gt in range(2)]

        with tc.tile_pool(name="ps_o", bufs=2, space="PSUM") as ps_o:
            for op in dve_rest:
                if op[0] == "sh":
                    shuffle_op(op[1], op[2])
                else:
                    m2_op(op[1], op[2], op[3])
            for c, w in enumerate(CHUNKS):
                j0 = COFF[c]
                pi = piece_of(j0, j0 + w)
                off = j0 - POFF[pi]
                o_ps = ps_o.tile([P, R + 1, 256], F32, tag="o", name=f"o{c}")
                for r in range(R):
                    nc.tensor.matmul(
                        o_ps[:, r, 0:w],
                        st[r][:, 0:P],
                        m2[(r, 0, pi)][:, off : off + w],
                        start=True,
                        stop=False,
                    )
                    nc.tensor.matmul(
                        o_ps[:, r, 0:w],
                        st[r][:, P:G],
                        m2[(r, 1, pi)][:, off : off + w],
                        start=False,
                        stop=True,
                    )
                nc.tensor.matmul(
                    o_ps[:, R, 0:w],
                    ident_h,
                    fb[:, j0 : j0 + w],
                    start=True,
                    stop=True,
                )
                o_sb = outp.tile([P, 256, R + 1], F32, tag="ob", name=f"ob{c}")
                o_view = o_sb[:, 0:w, :].rearrange("p j c -> p c j")
                nc.scalar.copy(o_view, o_ps[:, :, 0:w])
                nc.sync.dma_start(out[:, j0 : j0 + w, :], o_sb[:, 0:w, :])

    _legalize_waits(nc)
    return nc


def _host_tables():
    """Packed fp16 header columns shared by all cores (cached)."""
    if "hdr" not in _cache:
        iota_g = np.tile(np.arange(G, dtype=np.float64), (P, 1)).astype(np.float16)
        ident_h = np.eye(P, dtype=np.float16)
        _cache["hdr"] = (iota_g, ident_h)
    return _cache["hdr"]


def kernel(F_genus: np.ndarray, labels: np.ndarray) -> np.ndarray:
    F_genus = np.ascontiguousarray(F_genus, dtype=np.float32)
    labels = np.ascontiguousarray(labels, dtype=np.int32)
    assert F_genus.shape == (B, N) and labels.shape == (R, N)

    iota_g, ident_h = _host_tables()
    # labT[p, r*NT + t] = labels[r, t*128 + p]
    labT = np.transpose(labels.reshape(R, NT, P), (2, 0, 1)).reshape(P, R * NT)
    iota_p = np.arange(P, dtype=np.float64)[:, None] + 128.0 * np.arange(2)[None, :]
    f32blk = np.concatenate(
        [iota_p, labT, np.eye(P)], axis=1
    ).astype(np.float32)  # [P, 322]
    f32_as_h = np.ascontiguousarray(f32blk).view(np.float16)  # [P, 644]
    hdr_cols = np.concatenate([iota_g, ident_h, f32_as_h], axis=1)  # [P, TAB]
    assert hdr_cols.shape == (P, TAB)
    lab4 = labels.astype(np.float16)

    in_maps = []
    for c in range(NCORES):
        Fc = F_genus[c * BL : (c + 1) * BL]
        # ft[i, t*128 + b] = Fc[b, t*128 + i]
        ft = np.ascontiguousarray(
            Fc.reshape(BL, NT, P).transpose(2, 1, 0).reshape(P, N)
        ).astype(np.float16)
        fbh = Fc.astype(np.float16)
        in_maps.append(
            {
                "ft_in": np.concatenate([hdr_cols, ft], axis=1),
                "fb_in": fbh,
                "lab4_in": lab4,
            }
        )

    # The first execution of a freshly compiled NEFF occasionally hits a
    # transient NRT_EXEC_UNIT_UNRECOVERABLE; a rebuild + retry recovers.
    last_err = None
    for attempt in range(3):
        try:
            if "nc" not in _cache:
                _cache["nc"] = _build_nc()
            res = run_bass_kernel_spmd(
                _cache["nc"], in_maps, core_ids=list(range(NCORES))
            )
            return np.concatenate([r["out"] for r in res.results], axis=0)
        except Exception as e:  # noqa: BLE001
            last_err = e
            _cache.pop("nc", None)
            import time as _time

            _time.sleep(3.0)
    raise last_err


# revision 40
# speedup vs baseline: 1.1772x; 1.0032x over previous
"""Trainium2 Bass kernel for nn_Encoder_23124103922122 (segment_reduce).

Math (per rank r of 6, labels lab_r[0..4095] in [0,256)):
    seg_r[b, g]  = sum_{i: lab_r[i]==g} F[b, i]          (segment sum)
    out[b, j, r] = seg_r[b, lab_r[j]]                     (gather back)
    out[b, j, 6] = F[b, j]                                (identity channel)

Cost-model-driven design; the per-core DMA device is the wall (~47us: 40.8us
of output stores + 5.8us of input loads at the modeled 360 GB/s):
  - Host pre-transposes F and sends it as fp16, plus all lookup tables packed
    into the head of the same tensor, so the FIRST DMA delivers everything
    stage 1 needs (no on-chip F transpose, input phase ~3 DMAs).
  - Pure-fp16 PE pipeline (masks are exact one-hot; F/seg rounding ~3e-4
    overall): fp16 moving operand = full PE rate; p-state warmup chain of
    dummy matmuls on a memset tile keeps the ramp clock hot from ~0.5us.
  - Stage 1: one-hot masks [i,g] built per (rank, i-tile): DVE majority
    (fp16 4x mode, 127ns each), ~2/9 on Pool, ~1/10 on ACT (abs+relu pair);
    tile-major matmul order so PE never waits on the streamed F load; each
    rank's seg->transpose->seg_T chain overlaps the next rank's tail.
    Each rank gets its own PSUM bank: co-resident accumulation groups with
    interleaved start flags corrupt each other on HW.
  - Stage 2: label rows broadcast to 128 partitions via DVE stream_shuffle
    on u32-bitcast pieces (no DMA broadcast - DMA has no spare bandwidth);
    m2 one-hot [g,j] pieces sized so early chunks' masks are ready first.
    Per j-chunk, 6 rank matmuls + 1 identity matmul write channel-major
    into one PSUM bank; ONE strided ACT copy interleaves PSUM->SBUF
    [128, w, 7]; stores stream on the SP queue (graduated widths: small
    first for an early start, small last for a short drain).

Sharding: data-parallel over batch B=1024 -> 8 cores x 128 rows. Labels &
tables replicated. No cross-device communication.

Cost-model timeline ~75.4us/core (baseline was 88.7us); measured rel err
~3e-4 vs the fp32 reference.

Note: walrus in this container accepts at most ONE sync-wait per instruction
(two on EventSemaphore); _legalize_waits() post-processes the Tile-scheduled
program to satisfy that. The BIR verifier also requires every written
location to have a reader - hence the wsink dummy output and the preamble
const consumption.
"""

import sys

if "/opt/trn_rl_repo" not in sys.path:
    sys.path.insert(0, "/opt/trn_rl_repo")

from contextlib import ExitStack

import numpy as np

import concourse.bass as bass
import concourse.mybir as mybir
import concourse.tile as tile
from concourse.bass import ts
from concourse.bass_utils import run_bass_kernel_spmd

B, N, R, G = 1024, 4096, 6, 256
NCORES = 8
BL = B // NCORES  # 128 batch rows per core
P = 128
NT = N // P  # 32 genus tiles
JC = 64  # stage-2 chunk width (64 j x 7 ch = 448 f32 in one PSUM bank)
NCH = N // JC  # 64 chunks
QW = 1024  # m2/lab_bc quarter width in j
NQ = N // QW  # 4
CQ = QW // JC  # 16 chunks per quarter
F32 = mybir.dt.float32
FP16 = mybir.dt.float16
U32 = mybir.dt.uint32
TAB = G + P + 2 * (2 + R * NT + P)  # 1028 fp16 cols of packed tables

_cache: dict = {}

# Engine -> prefix of the semaphore names its compute instructions increment.
_ENGINE_SEM_PREFIX = {
    mybir.EngineType.PE: "PE",
    mybir.EngineType.DVE: "DVE",
    mybir.EngineType.Activation: "Activation",
    mybir.EngineType.Pool: "Pool",
    mybir.EngineType.SP: "SP",
}


def _legalize_waits(nc):
    """Walrus only accepts 1 sync-wait per instruction (2 on EventSemaphore),
    but the Tile scheduler can emit more. Post-pass:
      1. drop waits on the instruction's own engine semaphore that are already
         satisfied by same-engine program order (compute completion is in-order
         and sem targets are absolute), and
      2. hoist remaining excess waits onto EventSemaphore carrier instructions
         inserted just before the instruction on the same engine.
    """
    ev_id = 0
    for f in nc.m.functions:
        for blk in f.blocks:
            insts = blk.instructions
            sem_incs: dict = {}  # (engine, sem_name) -> cumulative inc in stream
            new_insts = []
            for inst in insts:
                si = inst.sync_info
                if si is not None and si.on_wait:
                    cap = 2 if isinstance(inst, mybir.InstEventSemaphore) else 1
                    eng = inst.engine
                    pfx = _ENGINE_SEM_PREFIX.get(eng)
                    kept = []
                    for w in si.on_wait:
                        sem_eng = w.ant_name.rsplit("_", 1)[0]
                        if (
                            pfx is not None
                            and sem_eng == pfx
                            and w.wait_mode == "sem-ge-imm"
                            and sem_incs.get((eng, w.ant_name), 0) >= w.wait_value
                        ):
                            continue  # satisfied by same-engine execution order
                        kept.append(w)
                    while len(kept) > cap:
                        ncarry = min(2, len(kept) - cap + 1)
                        carry, kept = kept[:ncarry], kept[ncarry:]
                        ev = mybir.InstEventSemaphore(
                            name=f"EVW-{ev_id}", ins=[], outs=[]
                        )
                        ev_id += 1
                        ev.engine = eng
                        ev.sync_info = mybir.SyncInfo(on_wait=carry, on_update=[])
                        new_insts.append(ev)
                    inst.sync_info = mybir.SyncInfo(
                        on_wait=kept, on_update=si.on_update
                    )
                si = inst.sync_info
                if si is not None:
                    for u in si.on_update:
                        if u.update_mode == "sem-inc":
                            key = (inst.engine, u.ant_name)
                            sem_incs[key] = sem_incs.get(key, 0) + u.update_value
                new_insts.append(inst)
            if len(new_insts) != len(insts):
                insts[:] = new_insts


# Per-rank stage-1 mask engine split (32 tiles): DVE carries most, Pool and
# ACT absorb the rest so DVE can also build the stage-2 masks in time.
_S1_POOL_T = {2, 6, 11, 16, 21, 26, 30}  # 7 per rank on Pool
_S1_ACT_T = {4, 14, 24}  # 3 per rank on ACT

# Stage-2 chunk-copy engine rotation (PSUM->SBUF interleave): per 8 chunks.
_COPY_ROT = ["act", "dve", "pool", "act", "dve", "act", "dve", "pool"]


def _build_nc():
    nc = bass.Bass("TRN2", debug=False, num_devices=NCORES)

    # ft_in layout: [tables header (TAB fp16 cols) | ft columns (N)]
    #   hdr[0:G] = iota_g fp16; hdr[G:G+P] = identity fp16;
    #   hdr[G+P : G+P+644] = f32 table bitcast as fp16 pairs:
    #       f32[0:2] iota_p, f32[2:194] labT, f32[194:322] identity f32
    ft_in = nc.dram_tensor("ft_in", [P, TAB + N], FP16, kind="ExternalInput").ap()
    fb_in = nc.dram_tensor("fb_in", [P, N], FP16, kind="ExternalInput").ap()
    lab4_in = nc.dram_tensor("lab4_in", [R, N], FP16, kind="ExternalInput").ap()
    out = nc.dram_tensor("out", [BL, N, R + 1], F32, kind="ExternalOutput").ap()
    # tiny sink for the PE-warmup results (BIR verifier needs every location read)
    wsink = nc.dram_tensor("wsink", [P, 8], F32, kind="ExternalOutput").ap()

    with ExitStack() as ctx:
        tc = ctx.enter_context(tile.TileContext(nc))

        const = ctx.enter_context(tc.tile_pool(name="const", bufs=1))
        fpool = ctx.enter_context(tc.tile_pool(name="fpool", bufs=1))
        lbc = ctx.enter_context(tc.tile_pool(name="lbc", bufs=6))
        m1p = ctx.enter_context(tc.tile_pool(name="m1p", bufs=32))
        mabsp = ctx.enter_context(tc.tile_pool(name="mabsp", bufs=6))
        m2p = ctx.enter_context(tc.tile_pool(name="m2p", bufs=18))
        segp = ctx.enter_context(tc.tile_pool(name="segp", bufs=1))
        outp = ctx.enter_context(tc.tile_pool(name="outp", bufs=4))

        # ---- constant tables first (tiny), then F streams ----
        from concourse.tile import add_dep_helper

        # one combined header+ft tensor: tables arrive with the very first
        # DMA; lab4 goes last (not needed until mid-stage-1)
        hdr = fpool.tile([P, TAB + N], FP16)
        HCUTS = [0, TAB + 512, TAB + 2048, TAB + N]
        d_ft = []
        for ci in range(len(HCUTS) - 1):
            a, b = HCUTS[ci], HCUTS[ci + 1]
            d_ft.append(nc.sync.dma_start(hdr[:, a:b], ft_in[:, a:b]))
            if ci:
                add_dep_helper(d_ft[ci].ins, d_ft[ci - 1].ins, reason="queue order")
        fb = fpool.tile([P, N], FP16)
        d_fb = nc.sync.dma_start(fb[:], fb_in)
        add_dep_helper(d_fb.ins, d_ft[-1].ins, reason="fb after ft")
        lab4 = const.tile([P, N], FP16)
        d_lab = [
            nc.sync.dma_start(lab4[32 * a : 32 * a + R, :], lab4_in)
            for a in range(4)
        ]
        add_dep_helper(d_lab[0].ins, d_fb.ins, reason="lab4 last")

        ft = hdr[:, TAB : TAB + N]
        f32v = hdr[:, G + P : TAB].bitcast(F32)  # [P, 322] f32 view
        iota_p = f32v[:, 0:2]
        labT = f32v[:, 2 : 2 + R * NT]
        ident_f = f32v[:, 2 + R * NT : 322]
        iota_g = hdr[:, 0:G]
        ident_h = hdr[:, G : G + P]

        # Prewarm: absorb const-DMA semaphores into engine clocks with cheap
        # ops so hot-loop instructions carry at most one sync wait each.
        wsrc = const.tile([P, P], FP16)
        nc.vector.memset(wsrc[:], 0)
        warm = const.tile([P, 8], F32)
        nc.vector.memset(warm[:], 0)
        # consume the preamble const tiles (BIR verifier wants readers)
        for _d, _v in (
            (mybir.dt.float32, 0.0),
            (mybir.dt.float32, 1.0),
            (mybir.dt.bfloat16, 1.0),
            (mybir.dt.uint8, 127),
        ):
            nc.vector.tensor_copy(warm[:, 4:5], nc.const_aps.aps[(_d, _v)])

        st = []  # seg_T fp16 per rank: st[r][g_loc, 128*gt + b]
        m2 = {}  # (r, gt, pi) -> [P, 1024] fp16 tile (first w cols valid)
        lab_bc = {}  # (r, pi) -> [P, 1024] fp16 broadcast label row

        # j-space piecing for lab_bc/m2: a small piece 0 (cheap to build on
        # DVE before the first store chunk), then 1024-wide pieces built
        # during the store phase.
        PIECES = [256] * 4 + [512] * 6  # sums to N
        POFF = [0]
        for w in PIECES:
            POFF.append(POFF[-1] + w)
        # store chunk widths (graduated: small first for early store start)
        CHUNKS = [64, 64, 128] + [256] * 14 + [128, 64, 64]  # sums to N
        COFF = [0]
        for w in CHUNKS:
            COFF.append(COFF[-1] + w)

        def piece_of(j0, j1):
            for pi in range(len(PIECES)):
                if POFF[pi] <= j0 and j1 <= POFF[pi + 1]:
                    return pi
            raise AssertionError((j0, j1))

        def shuffle_op(r, pi):
            w = PIECES[pi]
            t_l = lbc.tile([P, 1024], FP16, tag=f"lbc{pi % 2}", name=f"lbc{r}_{pi}")
            nc.vector.stream_shuffle(
                t_l[:, 0:w].bitcast(U32),
                lab4[:, POFF[pi] : POFF[pi] + w].bitcast(U32),
                [r] * 32,
            )
            lab_bc[(r, pi)] = t_l

        def m2_op(r, gt, pi):
            w = PIECES[pi]
            t_m = m2p.tile([P, 1024], FP16, tag=f"m2_{pi % 2}", name=f"m2_{r}_{gt}_{pi}")
            nc.vector.tensor_scalar(
                t_m[:, 0:w],
                lab_bc[(r, pi)][:, 0:w],
                iota_p[:, gt : gt + 1],
                None,
                op0=mybir.AluOpType.is_equal,
            )
            m2[(r, gt, pi)] = t_m

        with tc.tile_pool(name="ps_seg", bufs=1, space="PSUM") as ps_seg, \
                tc.tile_pool(name="ps_tr", bufs=2, space="PSUM") as ps_tr:
            # PE p-state warmup: small dependency-free matmuls start the ramp
            # clock before the real stage-1 work arrives.
            wps = [
                ps_tr.tile([P, G], F32, tag="tr", name=f"w{i}") for i in range(2)
            ]
            for i in range(28):
                nc.tensor.matmul(
                    wps[i % 2][:, 0:64],
                    wsrc[:],
                    wsrc[:, 0:64],
                    start=True,
                    stop=True,
                    skip_group_check=True,
                )
            # absorb the warm chain (verifier needs readers)
            nc.scalar.copy(warm[:, 6:7], wps[0][:, 0:1])
            nc.scalar.copy(warm[:, 7:8], wps[1][:, 0:1])
            nc.scalar.dma_start(wsink, warm[:])

            seg_tiles = [
                ps_seg.tile([P, G], F32, tag=f"segp{r}", name=f"segp{r}")
                for r in range(R)
            ]
            seg_ps = [seg_tiles[r][:] for r in range(R)]

            def s1_mask(r, t):
                col = r * NT + t
                idx = t * R + r
                m1 = m1p.tile([P, G], FP16, tag="m1")
                if idx % 10 == 3:  # ACT via abs+relu (exact for ints)
                    tmp = mabsp.tile([P, G], F32, tag="mabs")
                    nc.scalar.activation(
                        tmp[:],
                        iota_g,
                        mybir.ActivationFunctionType.Abs,
                        bias=labT[:, col : col + 1],
                        scale=-1.0,
                    )
                    nc.scalar.activation(
                        m1[:],
                        tmp[:],
                        mybir.ActivationFunctionType.Relu,
                        bias=1.0,
                        scale=-1.0,
                    )
                elif idx % 9 in (1, 5):  # Pool
                    nc.gpsimd.tensor_scalar(
                        m1[:],
                        iota_g,
                        labT[:, col : col + 1],
                        None,
                        op0=mybir.AluOpType.is_equal,
                    )
                else:  # DVE (4x fp16 mode)
                    nc.vector.tensor_scalar(
                        m1[:],
                        iota_g,
                        labT[:, col : col + 1],
                        None,
                        op0=mybir.AluOpType.is_equal,
                    )
                return m1

            def s1_matmul(r, t, m1):
                nc.tensor.matmul(
                    seg_ps[r],
                    ft[:, ts(t, P)],
                    m1[:],
                    start=(t == 0),
                    stop=(t == NT - 1),
                )

            # DVE inserts during stage 1: piece-0 shuffles + m2 (cheap) and
            # piece-1 shuffles, spread across the tile loop.
            inserts = (
                [("sh", r, 0) for r in range(R)]
                + [("m2", r, gt, 0) for r in range(R) for gt in range(2)]
                + [("sh", r, 1) for r in range(R)]
                + [("m2", r, gt, 1) for r in range(R) for gt in range(2)]
            )
            # ~1.4 ops per tile from tile 2 on
            ins_at = {}
            for k, op in enumerate(inserts):
                ins_at.setdefault(2 + (2 * k) // 3, []).append(op)

            # tile-major over t=0..27 (tile t only needs its own ft quarter,
            # so PE never stalls on the F load), then per-rank finish for
            # t=28..31 so each rank's seg->seg_T chain overlaps the next
            # rank's matmuls.
            for t in range(NT - 4):
                for r in range(R):
                    s1_matmul(r, t, s1_mask(r, t))
                for op in ins_at.get(t, ()):
                    if op[0] == "sh":
                        shuffle_op(op[1], op[2])
                    else:
                        m2_op(op[1], op[2], op[3])
            # finish all remaining matmuls rank-by-rank, chains as each
            # rank completes; the copies ride the idle Pool engine so ACT is
            # free for the first chunk copies.
            pts = []
            for r in range(R):
                for t in range(NT - 4, NT):
                    s1_matmul(r, t, s1_mask(r, t))
                s_sb = segp.tile([P, G], F32, tag=f"ssb{r}", name=f"ssb{r}")
                nc.scalar.copy(s_sb[:], seg_ps[r])
                pt = ps_tr.tile([P, G], F32, tag="tr")
                for gt in range(2):
                    nc.tensor.transpose(
                        pt[:, ts(gt, P)], s_sb[:, ts(gt, P)], ident_f
                    )
                pts.append(pt)
                st_r = segp.tile([P, G], FP16, tag=f"st{r}", name=f"st{r}")
                nc.scalar.copy(st_r[:], pt[:])
                st.append(st_r)

        # ---- stage 2: per chunk, channel-major PSUM, one interleave copy
        # (ACT/Pool alternating), store on the SP queue. DVE runs the
        # remaining piece builds as one serial stream; it stays ahead of the
        # 2.55us/chunk store rate. ----
        dve_rest = []
        for pi in range(2, len(PIECES)):
            dve_rest += [("sh", r, pi) for r in range(R)]
            dve_rest += [("m2", r, gt, pi) for r in range(R) for gt in range(2)]

        with tc.tile_pool(name="ps_o", bufs=2, space="PSUM") as ps_o:
            for op in dve_rest:
                if op[0] == "sh":
                    shuffle_op(op[1], op[2])
                else:
                    m2_op(op[1], op[2], op[3])
            for c, w in enumerate(CHUNKS):
                j0 = COFF[c]
                pi = piece_of(j0, j0 + w)
                off = j0 - POFF[pi]
                o_ps = ps_o.tile([P, R + 1, 256], F32, tag="o", name=f"o{c}")
                for r in range(R):
                    nc.tensor.matmul(
                        o_ps[:, r, 0:w],
                        st[r][:, 0:P],
                        m2[(r, 0, pi)][:, off : off + w],
                        start=True,
                        stop=False,
                    )
                    nc.tensor.matmul(
                        o_ps[:, r, 0:w],
                        st[r][:, P:G],
                        m2[(r, 1, pi)][:, off : off + w],
                        start=False,
                        stop=True,
                    )
                nc.tensor.matmul(
                    o_ps[:, R, 0:w],
                    ident_h,
                    fb[:, j0 : j0 + w],
                    start=True,
                    stop=True,
                )
                o_sb = outp.tile([P, 256, R + 1], F32, tag="ob", name=f"ob{c}")
                o_view = o_sb[:, 0:w, :].rearrange("p j c -> p c j")
                nc.scalar.copy(o_view, o_ps[:, :, 0:w])
                nc.sync.dma_start(out[:, j0 : j0 + w, :], o_sb[:, 0:w, :])

    _legalize_waits(nc)
    return nc


def _host_tables():
    """Packed fp16 header columns shared by all cores (cached)."""
    if "hdr" not in _cache:
        iota_g = np.tile(np.arange(G, dtype=np.float64), (P, 1)).astype(np.float16)
        ident_h = np.eye(P, dtype=np.float16)
        _cache["hdr"] = (iota_g, ident_h)
    return _cache["hdr"]


def kernel(F_genus: np.ndarray, labels: np.ndarray) -> np.ndarray:
    F_genus = np.ascontiguousarray(F_genus, dtype=np.float32)
    labels = np.ascontiguousarray(labels, dtype=np.int32)
    assert F_genus.shape == (B, N) and labels.shape == (R, N)

    iota_g, ident_h = _host_tables()
    # labT[p, r*NT + t] = labels[r, t*128 + p]
    labT = np.transpose(labels.reshape(R, NT, P), (2, 0, 1)).reshape(P, R * NT)
    iota_p = np.arange(P, dtype=np.float64)[:, None] + 128.0 * np.arange(2)[None, :]
    f32blk = np.concatenate(
        [iota_p, labT, np.eye(P)], axis=1
    ).astype(np.float32)  # [P, 322]
    f32_as_h = np.ascontiguousarray(f32blk).view(np.float16)  # [P, 644]
    hdr_cols = np.concatenate([iota_g, ident_h, f32_as_h], axis=1)  # [P, TAB]
    assert hdr_cols.shape == (P, TAB)
    lab4 = labels.astype(np.float16)

    in_maps = []
    for c in range(NCORES):
        Fc = F_genus[c * BL : (c + 1) * BL]
        # ft[i, t*128 + b] = Fc[b, t*128 + i]
        ft = np.ascontiguousarray(
            Fc.reshape(BL, NT, P).transpose(2, 1, 0).reshape(P, N)
        ).astype(np.float16)
        fbh = Fc.astype(np.float16)
        in_maps.append(
            {
                "ft_in": np.concatenate([hdr_cols, ft], axis=1),
                "fb_in": fbh,
                "lab4_in": lab4,
            }
        )

    # The first execution of a freshly compiled NEFF occasionally hits a
    # transient NRT_EXEC_UNIT_UNRECOVERABLE; a rebuild + retry recovers.
    last_err = None
    for attempt in range(3):
        try:
            if "nc" not in _cache:
                _cache["nc"] = _build_nc()
            res = run_bass_kernel_spmd(
                _cache["nc"], in_maps, core_ids=list(range(NCORES))
            )
            return np.concatenate([r["out"] for r in res.results], axis=0)
        except Exception as e:  # noqa: BLE001
            last_err = e
            _cache.pop("nc", None)
            import time as _time

            _time.sleep(3.0)
    raise last_err


# revision 46
# speedup vs baseline: 1.1796x; 1.0021x over previous
"""Trainium2 Bass kernel for nn_Encoder_23124103922122 (segment_reduce).

Math (per rank r of 6, labels lab_r[0..4095] in [0,256)):
    seg_r[b, g]  = sum_{i: lab_r[i]==g} F[b, i]          (segment sum)
    out[b, j, r] = seg_r[b, lab_r[j]]                     (gather back)
    out[b, j, 6] = F[b, j]                                (identity channel)

Cost-model-driven design; the per-core DMA device is the wall (~47us: 40.8us
of output stores + 5.8us of input loads at the modeled 360 GB/s):
  - Host pre-transposes F and sends it as fp16, plus all lookup tables packed
    into the head of the same tensor, so the FIRST DMA delivers everything
    stage 1 needs (no on-chip F transpose, input phase ~3 DMAs).
  - Pure-fp16 PE pipeline (masks are exact one-hot; F/seg rounding ~3e-4
    overall): fp16 moving operand = full PE rate; p-state warmup chain of
    dummy matmuls on a memset tile keeps the ramp clock hot from ~0.5us.
  - Stage 1: one-hot masks [i,g] built per (rank, i-tile): DVE majority
    (fp16 4x mode, 127ns each), ~2/9 on Pool, ~1/10 on ACT (abs+relu pair);
    tile-major matmul order so PE never waits on the streamed F load; each
    rank's seg->transpose->seg_T chain overlaps the next rank's tail.
    Each rank gets its own PSUM bank: co-resident accumulation groups with
    interleaved start flags corrupt each other on HW.
  - Stage 2: label rows broadcast to 128 partitions via DVE stream_shuffle
    on u32-bitcast pieces (no DMA broadcast - DMA has no spare bandwidth);
    m2 one-hot [g,j] pieces sized so early chunks' masks are ready first.
    Per j-chunk, 6 rank matmuls + 1 identity matmul write channel-major
    into one PSUM bank; ONE strided ACT copy interleaves PSUM->SBUF
    [128, w, 7]; stores stream on the SP queue (graduated widths: small
    first for an early start, small last for a short drain).

Sharding: data-parallel over batch B=1024 -> 8 cores x 128 rows. Labels &
tables replicated. No cross-device communication.

Cost-model timeline ~75.4us/core (baseline was 88.7us); measured rel err
~3e-4 vs the fp32 reference.

Note: walrus in this container accepts at most ONE sync-wait per instruction
(two on EventSemaphore); _legalize_waits() post-processes the Tile-scheduled
program to satisfy that. The BIR verifier also requires every written
location to have a reader - hence the wsink dummy output and the preamble
const consumption.
"""

import sys

if "/opt/trn_rl_repo" not in sys.path:
    sys.path.insert(0, "/opt/trn_rl_repo")

from contextlib import ExitStack

import numpy as np

import concourse.bass as bass
import concourse.mybir as mybir
import concourse.tile as tile
from concourse.bass import ts
from concourse.bass_utils import run_bass_kernel_spmd

B, N, R, G = 1024, 4096, 6, 256
NCORES = 8
BL = B // NCORES  # 128 batch rows per core
P = 128
NT = N // P  # 32 genus tiles
JC = 64  # stage-2 chunk width (64 j x 7 ch = 448 f32 in one PSUM bank)
NCH = N // JC  # 64 chunks
QW = 1024  # m2/lab_bc quarter width in j
NQ = N // QW  # 4
CQ = QW // JC  # 16 chunks per quarter
F32 = mybir.dt.float32
FP16 = mybir.dt.float16
U32 = mybir.dt.uint32
TAB = G + P + 2 * (2 + R * NT + P)  # 1028 fp16 cols of packed tables

_cache: dict = {}

# Engine -> prefix of the semaphore names its compute instructions increment.
_ENGINE_SEM_PREFIX = {
    mybir.EngineType.PE: "PE",
    mybir.EngineType.DVE: "DVE",
    mybir.EngineType.Activation: "Activation",
    mybir.EngineType.Pool: "Pool",
    mybir.EngineType.SP: "SP",
}


def _legalize_waits(nc):
    """Walrus only accepts 1 sync-wait per instruction (2 on EventSemaphore),
    but the Tile scheduler can emit more. Post-pass:
      1. drop waits on the instruction's own engine semaphore that are already
         satisfied by same-engine program order (compute completion is in-order
         and sem targets are absolute), and
      2. hoist remaining excess waits onto EventSemaphore carrier instructions
         inserted just before the instruction on the same engine.
    """
    ev_id = 0
    for f in nc.m.functions:
        for blk in f.blocks:
            insts = blk.instructions
            sem_incs: dict = {}  # (engine, sem_name) -> cumulative inc in stream
            new_insts = []
            for inst in insts:
                si = inst.sync_info
                if si is not None and si.on_wait:
                    cap = 2 if isinstance(inst, mybir.InstEventSemaphore) else 1
                    eng = inst.engine
                    pfx = _ENGINE_SEM_PREFIX.get(eng)
                    kept = []
                    for w in si.on_wait:
                        sem_eng = w.ant_name.rsplit("_", 1)[0]
                        if (
                            pfx is not None
                            and sem_eng == pfx
                            and w.wait_mode == "sem-ge-imm"
                            and sem_incs.get((eng, w.ant_name), 0) >= w.wait_value
                        ):
                            continue  # satisfied by same-engine execution order
                        kept.append(w)
                    while len(kept) > cap:
                        ncarry = min(2, len(kept) - cap + 1)
                        carry, kept = kept[:ncarry], kept[ncarry:]
                        ev = mybir.InstEventSemaphore(
                            name=f"EVW-{ev_id}", ins=[], outs=[]
                        )
                        ev_id += 1
                        ev.engine = eng
                        ev.sync_info = mybir.SyncInfo(on_wait=carry, on_update=[])
                        new_insts.append(ev)
                    inst.sync_info = mybir.SyncInfo(
                        on_wait=kept, on_update=si.on_update
                    )
                si = inst.sync_info
                if si is not None:
                    for u in si.on_update:
                        if u.update_mode == "sem-inc":
                            key = (inst.engine, u.ant_name)
                            sem_incs[key] = sem_incs.get(key, 0) + u.update_value
                new_insts.append(inst)
            if len(new_insts) != len(insts):
                insts[:] = new_insts


# Per-rank stage-1 mask engine split (32 tiles): DVE carries most, Pool and
# ACT absorb the rest so DVE can also build the stage-2 masks in time.
_S1_POOL_T = {2, 6, 11, 16, 21, 26, 30}  # 7 per rank on Pool
_S1_ACT_T = {4, 14, 24}  # 3 per rank on ACT

# Stage-2 chunk-copy engine rotation (PSUM->SBUF interleave): per 8 chunks.
_COPY_ROT = ["act", "dve", "pool", "act", "dve", "act", "dve", "pool"]


def _build_nc():
    nc = bass.Bass("TRN2", debug=False, num_devices=NCORES)

    # ft_in layout: [tables header (TAB fp16 cols) | ft columns (N)]
    #   hdr[0:G] = iota_g fp16; hdr[G:G+P] = identity fp16;
    #   hdr[G+P : G+P+644] = f32 table bitcast as fp16 pairs:
    #       f32[0:2] iota_p, f32[2:194] labT, f32[194:322] identity f32
    ft_in = nc.dram_tensor("ft_in", [P, TAB + N], FP16, kind="ExternalInput").ap()
    fb_in = nc.dram_tensor("fb_in", [P, N], FP16, kind="ExternalInput").ap()
    lab4_in = nc.dram_tensor("lab4_in", [R, N], FP16, kind="ExternalInput").ap()
    out = nc.dram_tensor("out", [BL, N, R + 1], F32, kind="ExternalOutput").ap()
    # tiny sink for the PE-warmup results (BIR verifier needs every location read)
    wsink = nc.dram_tensor("wsink", [P, 8], F32, kind="ExternalOutput").ap()

    with ExitStack() as ctx:
        tc = ctx.enter_context(tile.TileContext(nc))

        const = ctx.enter_context(tc.tile_pool(name="const", bufs=1))
        fpool = ctx.enter_context(tc.tile_pool(name="fpool", bufs=1))
        lbc = ctx.enter_context(tc.tile_pool(name="lbc", bufs=6))
        m1p = ctx.enter_context(tc.tile_pool(name="m1p", bufs=32))
        mabsp = ctx.enter_context(tc.tile_pool(name="mabsp", bufs=6))
        m2p = ctx.enter_context(tc.tile_pool(name="m2p", bufs=18))
        segp = ctx.enter_context(tc.tile_pool(name="segp", bufs=1))
        outp = ctx.enter_context(tc.tile_pool(name="outp", bufs=4))

        # ---- constant tables first (tiny), then F streams ----
        from concourse.tile import add_dep_helper

        # one combined header+ft tensor: tables arrive with the very first
        # DMA; lab4 goes last (not needed until mid-stage-1)
        hdr = fpool.tile([P, TAB + N], FP16)
        HCUTS = [0, TAB + 512, TAB + 2048, TAB + N]
        d_ft = []
        for ci in range(len(HCUTS) - 1):
            a, b = HCUTS[ci], HCUTS[ci + 1]
            d_ft.append(nc.sync.dma_start(hdr[:, a:b], ft_in[:, a:b]))
        lab4 = const.tile([P, N], FP16)
        d_lab = [
            nc.sync.dma_start(lab4[32 * a : 32 * a + R, :], lab4_in)
            for a in range(4)
        ]
        fb = fpool.tile([P, N], FP16)
        d_fb = nc.sync.dma_start(fb[:], fb_in)
        add_dep_helper(d_fb.ins, d_ft[-1].ins, reason="fb last")

        ft = hdr[:, TAB : TAB + N]
        f32v = hdr[:, G + P : TAB].bitcast(F32)  # [P, 322] f32 view
        iota_p = f32v[:, 0:2]
        labT = f32v[:, 2 : 2 + R * NT]
        ident_f = f32v[:, 2 + R * NT : 322]
        iota_g = hdr[:, 0:G]
        ident_h = hdr[:, G : G + P]

        # Prewarm: absorb const-DMA semaphores into engine clocks with cheap
        # ops so hot-loop instructions carry at most one sync wait each.
        wsrc = const.tile([P, P], FP16)
        nc.vector.memset(wsrc[:], 0)
        warm = const.tile([P, 8], F32)
        nc.vector.memset(warm[:], 0)
        # consume the preamble const tiles (BIR verifier wants readers)
        for _d, _v in (
            (mybir.dt.float32, 0.0),
            (mybir.dt.float32, 1.0),
            (mybir.dt.bfloat16, 1.0),
            (mybir.dt.uint8, 127),
        ):
            nc.vector.tensor_copy(warm[:, 4:5], nc.const_aps.aps[(_d, _v)])

        st = []  # seg_T fp16 per rank: st[r][g_loc, 128*gt + b]
        m2 = {}  # (r, gt, pi) -> [P, 1024] fp16 tile (first w cols valid)
        lab_bc = {}  # (r, pi) -> [P, 1024] fp16 broadcast label row

        # j-space piecing for lab_bc/m2: a small piece 0 (cheap to build on
        # DVE before the first store chunk), then 1024-wide pieces built
        # during the store phase.
        PIECES = [256] * 4 + [512] * 6  # sums to N
        POFF = [0]
        for w in PIECES:
            POFF.append(POFF[-1] + w)
        # store chunk widths (graduated: small first for early store start)
        CHUNKS = [64, 64, 128] + [256] * 14 + [128, 64, 64]  # sums to N
        COFF = [0]
        for w in CHUNKS:
            COFF.append(COFF[-1] + w)

        def piece_of(j0, j1):
            for pi in range(len(PIECES)):
                if POFF[pi] <= j0 and j1 <= POFF[pi + 1]:
                    return pi
            raise AssertionError((j0, j1))

        def shuffle_op(r, pi):
            w = PIECES[pi]
            t_l = lbc.tile([P, 1024], FP16, tag=f"lbc{pi % 2}", name=f"lbc{r}_{pi}")
            nc.vector.stream_shuffle(
                t_l[:, 0:w].bitcast(U32),
                lab4[:, POFF[pi] : POFF[pi] + w].bitcast(U32),
                [r] * 32,
            )
            lab_bc[(r, pi)] = t_l

        def m2_op(r, gt, pi):
            w = PIECES[pi]
            t_m = m2p.tile([P, 1024], FP16, tag=f"m2_{pi % 2}", name=f"m2_{r}_{gt}_{pi}")
            nc.vector.tensor_scalar(
                t_m[:, 0:w],
                lab_bc[(r, pi)][:, 0:w],
                iota_p[:, gt : gt + 1],
                None,
                op0=mybir.AluOpType.is_equal,
            )
            m2[(r, gt, pi)] = t_m

        with tc.tile_pool(name="ps_seg", bufs=1, space="PSUM") as ps_seg, \
                tc.tile_pool(name="ps_tr", bufs=2, space="PSUM") as ps_tr:
            # PE p-state warmup: small dependency-free matmuls start the ramp
            # clock before the real stage-1 work arrives.
            wps = [
                ps_tr.tile([P, G], F32, tag="tr", name=f"w{i}") for i in range(2)
            ]
            for i in range(28):
                nc.tensor.matmul(
                    wps[i % 2][:, 0:64],
                    wsrc[:],
                    wsrc[:, 0:64],
                    start=True,
                    stop=True,
                    skip_group_check=True,
                )
            # absorb the warm chain (verifier needs readers)
            nc.scalar.copy(warm[:, 6:7], wps[0][:, 0:1])
            nc.scalar.copy(warm[:, 7:8], wps[1][:, 0:1])
            nc.scalar.dma_start(wsink, warm[:])

            seg_tiles = [
                ps_seg.tile([P, G], F32, tag=f"segp{r}", name=f"segp{r}")
                for r in range(R)
            ]
            seg_ps = [seg_tiles[r][:] for r in range(R)]

            def s1_mask(r, t):
                col = r * NT + t
                idx = t * R + r
                m1 = m1p.tile([P, G], FP16, tag="m1")
                if idx % 10 == 3:  # ACT via abs+relu (exact for ints)
                    tmp = mabsp.tile([P, G], F32, tag="mabs")
                    nc.scalar.activation(
                        tmp[:],
                        iota_g,
                        mybir.ActivationFunctionType.Abs,
                        bias=labT[:, col : col + 1],
                        scale=-1.0,
                    )
                    nc.scalar.activation(
                        m1[:],
                        tmp[:],
                        mybir.ActivationFunctionType.Relu,
                        bias=1.0,
                        scale=-1.0,
                    )
                elif idx % 9 in (1, 5):  # Pool
                    nc.gpsimd.tensor_scalar(
                        m1[:],
                        iota_g,
                        labT[:, col : col + 1],
                        None,
                        op0=mybir.AluOpType.is_equal,
                    )
                else:  # DVE (4x fp16 mode)
                    nc.vector.tensor_scalar(
                        m1[:],
                        iota_g,
                        labT[:, col : col + 1],
                        None,
                        op0=mybir.AluOpType.is_equal,
                    )
                return m1

            def s1_matmul(r, t, m1):
                nc.tensor.matmul(
                    seg_ps[r],
                    ft[:, ts(t, P)],
                    m1[:],
                    start=(t == 0),
                    stop=(t == NT - 1),
                )

            # DVE inserts during stage 1: piece-0 shuffles + m2 (cheap) and
            # piece-1 shuffles, spread across the tile loop.
            ins_at = {}

            # tile-major over t=0..27 (tile t only needs its own ft quarter,
            # so PE never stalls on the F load), then per-rank finish for
            # t=28..31 so each rank's seg->seg_T chain overlaps the next
            # rank's matmuls.
            for t in range(NT - 4):
                for r in range(R):
                    s1_matmul(r, t, s1_mask(r, t))
                for op in ins_at.get(t, ()):
                    if op[0] == "sh":
                        shuffle_op(op[1], op[2])
                    else:
                        m2_op(op[1], op[2], op[3])
            # finish all remaining matmuls rank-by-rank, chains as each
            # rank completes; the copies ride the idle Pool engine so ACT is
            # free for the first chunk copies.
            pts = []
            for r in range(R):
                for t in range(NT - 4, NT):
                    s1_matmul(r, t, s1_mask(r, t))
                s_sb = segp.tile([P, G], F32, tag=f"ssb{r}", name=f"ssb{r}")
                nc.scalar.copy(s_sb[:], seg_ps[r])
                pt = ps_tr.tile([P, G], F32, tag="tr")
                for gt in range(2):
                    nc.tensor.transpose(
                        pt[:, ts(gt, P)], s_sb[:, ts(gt, P)], ident_f
                    )
                pts.append(pt)
                st_r = segp.tile([P, G], FP16, tag=f"st{r}", name=f"st{r}")
                nc.scalar.copy(st_r[:], pt[:])
                st.append(st_r)

        # ---- stage 2: per chunk, channel-major PSUM, one interleave copy
        # (ACT/Pool alternating), store on the SP queue. DVE runs the
        # remaining piece builds as one serial stream; it stays ahead of the
        # 2.55us/chunk store rate. ----
        dve_rest = []
        for pi in range(2):
            dve_rest += [("sh", r, pi) for r in range(R)]
            dve_rest += [("m2", r, gt, pi) for r in range(R) for gt in range(2)]
        for pi in range(2, len(PIECES)):
            dve_rest += [("sh", r, pi) for r in range(R)]
            dve_rest += [("m2", r, gt, pi) for r in range(R) for gt in range(2)]

        with tc.tile_pool(name="ps_o", bufs=2, space="PSUM") as ps_o:
            for op in dve_rest:
                if op[0] == "sh":
                    shuffle_op(op[1], op[2])
                else:
                    m2_op(op[1], op[2], op[3])
            for c, w in enumerate(CHUNKS):
                j0 = COFF[c]
                pi = piece_of(j0, j0 + w)
                off = j0 - POFF[pi]
                o_ps = ps_o.tile([P, R + 1, 256], F32, tag="o", name=f"o{c}")
                for r in range(R):
                    nc.tensor.matmul(
                        o_ps[:, r, 0:w],
                        st[r][:, 0:P],
                        m2[(r, 0, pi)][:, off : off + w],
                        start=True,
                        stop=False,
                    )
                    nc.tensor.matmul(
                        o_ps[:, r, 0:w],
                        st[r][:, P:G],
                        m2[(r, 1, pi)][:, off : off + w],
                        start=False,
                        stop=True,
                    )
                nc.tensor.matmul(
                    o_ps[:, R, 0:w],
                    ident_h,
                    fb[:, j0 : j0 + w],
                    start=True,
                    stop=True,
                )
                o_sb = outp.tile([P, 256, R + 1], F32, tag="ob", name=f"ob{c}")
                o_view = o_sb[:, 0:w, :].rearrange("p j c -> p c j")
                nc.scalar.copy(o_view, o_ps[:, :, 0:w])
                nc.sync.dma_start(out[:, j0 : j0 + w, :], o_sb[:, 0:w, :])

    _legalize_waits(nc)
    return nc


def _host_tables():
    """Packed fp16 header columns shared by all cores (cached)."""
    if "hdr" not in _cache:
        iota_g = np.tile(np.arange(G, dtype=np.float64), (P, 1)).astype(np.float16)
        ident_h = np.eye(P, dtype=np.float16)
        _cache["hdr"] = (iota_g, ident_h)
    return _cache["hdr"]


def kernel(F_genus: np.ndarray, labels: np.ndarray) -> np.ndarray:
    F_genus = np.ascontiguousarray(F_genus, dtype=np.float32)
    labels = np.ascontiguousarray(labels, dtype=np.int32)
    assert F_genus.shape == (B, N) and labels.shape == (R, N)

    iota_g, ident_h = _host_tables()
    # labT[p, r*NT + t] = labels[r, t*128 + p]
    labT = np.transpose(labels.reshape(R, NT, P), (2, 0, 1)).reshape(P, R * NT)
    iota_p = np.arange(P, dtype=np.float64)[:, None] + 128.0 * np.arange(2)[None, :]
    f32blk = np.concatenate(
        [iota_p, labT, np.eye(P)], axis=1
    ).astype(np.float32)  # [P, 322]
    f32_as_h = np.ascontiguousarray(f32blk).view(np.float16)  # [P, 644]
    hdr_cols = np.concatenate([iota_g, ident_h, f32_as_h], axis=1)  # [P, TAB]
    assert hdr_cols.shape == (P, TAB)
    lab4 = labels.astype(np.float16)

    in_maps = []
    for c in range(NCORES):
        Fc = F_genus[c * BL : (c + 1) * BL]
        # ft[i, t*128 + b] = Fc[b, t*128 + i]
        ft = np.ascontiguousarray(
            Fc.reshape(BL, NT, P).transpose(2, 1, 0).reshape(P, N)
        ).astype(np.float16)
        fbh = Fc.astype(np.float16)
        in_maps.append(
            {
                "ft_in": np.concatenate([hdr_cols, ft], axis=1),
                "fb_in": fbh,
                "lab4_in": lab4,
            }
        )

    # The first execution of a freshly compiled NEFF occasionally hits a
    # transient NRT_EXEC_UNIT_UNRECOVERABLE; a rebuild + retry recovers.
    last_err = None
    for attempt in range(3):
        try:
            if "nc" not in _cache:
                _cache["nc"] = _build_nc()
            res = run_bass_kernel_spmd(
                _cache["nc"], in_maps, core_ids=list(range(NCORES))
            )
            return np.concatenate([r["out"] for r in res.results], axis=0)
        except Exception as e:  # noqa: BLE001
            last_err = e
            _cache.pop("nc", None)
            import time as _time

            _time.sleep(3.0)
    raise last_err


# revision 54
# speedup vs baseline: 1.1831x; 1.0030x over previous
"""Trainium2 Bass kernel for nn_Encoder_23124103922122 (segment_reduce).

Math (per rank r of 6, labels lab_r[0..4095] in [0,256)):
    seg_r[b, g]  = sum_{i: lab_r[i]==g} F[b, i]          (segment sum)
    out[b, j, r] = seg_r[b, lab_r[j]]                     (gather back)
    out[b, j, 6] = F[b, j]                                (identity channel)

Cost-model-driven design; the per-core DMA device is the wall (~47us: 40.8us
of output stores + 5.8us of input loads at the modeled 360 GB/s):
  - Host pre-transposes F and sends it as fp16, plus all lookup tables packed
    into the head of the same tensor, so the FIRST DMA delivers everything
    stage 1 needs (no on-chip F transpose, input phase ~3 DMAs).
  - Pure-fp16 PE pipeline (masks are exact one-hot; F/seg rounding ~3e-4
    overall): fp16 moving operand = full PE rate; p-state warmup chain of
    dummy matmuls on a memset tile keeps the ramp clock hot from ~0.5us.
  - Stage 1: one-hot masks [i,g] built per (rank, i-tile): DVE majority
    (fp16 4x mode, 127ns each), ~2/9 on Pool, ~1/10 on ACT (abs+relu pair);
    tile-major matmul order so PE never waits on the streamed F load; each
    rank's seg->transpose->seg_T chain overlaps the next rank's tail.
    Each rank gets its own PSUM bank: co-resident accumulation groups with
    interleaved start flags corrupt each other on HW.
  - Stage 2: label rows broadcast to 128 partitions via DVE stream_shuffle
    on u32-bitcast pieces (no DMA broadcast - DMA has no spare bandwidth);
    m2 one-hot [g,j] pieces sized so early chunks' masks are ready first.
    Per j-chunk, 6 rank matmuls + 1 identity matmul write channel-major
    into one PSUM bank; ONE strided ACT copy interleaves PSUM->SBUF
    [128, w, 7]; stores stream on the SP queue (graduated widths: small
    first for an early start, small last for a short drain).

Sharding: data-parallel over batch B=1024 -> 8 cores x 128 rows. Labels &
tables replicated. No cross-device communication.

Cost-model timeline ~75.4us/core (baseline was 88.7us); measured rel err
~3e-4 vs the fp32 reference.

Note: walrus in this container accepts at most ONE sync-wait per instruction
(two on EventSemaphore); _legalize_waits() post-processes the Tile-scheduled
program to satisfy that. The BIR verifier also requires every written
location to have a reader - hence the wsink dummy output and the preamble
const consumption.
"""

import sys

if "/opt/trn_rl_repo" not in sys.path:
    sys.path.insert(0, "/opt/trn_rl_repo")

from contextlib import ExitStack

import numpy as np

import concourse.bass as bass
import concourse.mybir as mybir
import concourse.tile as tile
from concourse.bass import ts
from concourse.bass_utils import run_bass_kernel_spmd

B, N, R, G = 1024, 4096, 6, 256
NCORES = 8
BL = B // NCORES  # 128 batch rows per core
P = 128
NT = N // P  # 32 genus tiles
JC = 64  # stage-2 chunk width (64 j x 7 ch = 448 f32 in one PSUM bank)
NCH = N // JC  # 64 chunks
QW = 1024  # m2/lab_bc quarter width in j
NQ = N // QW  # 4
CQ = QW // JC  # 16 chunks per quarter
F32 = mybir.dt.float32
FP16 = mybir.dt.float16
U32 = mybir.dt.uint32
TAB = G + P + 2 * (2 + R * NT + P)  # 1028 fp16 cols of packed tables

_cache: dict = {}

# Engine -> prefix of the semaphore names its compute instructions increment.
_ENGINE_SEM_PREFIX = {
    mybir.EngineType.PE: "PE",
    mybir.EngineType.DVE: "DVE",
    mybir.EngineType.Activation: "Activation",
    mybir.EngineType.Pool: "Pool",
    mybir.EngineType.SP: "SP",
}


def _legalize_waits(nc):
    """Walrus only accepts 1 sync-wait per instruction (2 on EventSemaphore),
    but the Tile scheduler can emit more. Post-pass:
      1. drop waits on the instruction's own engine semaphore that are already
         satisfied by same-engine program order (compute completion is in-order
         and sem targets are absolute), and
      2. hoist remaining excess waits onto EventSemaphore carrier instructions
         inserted just before the instruction on the same engine.
    """
    ev_id = 0
    for f in nc.m.functions:
        for blk in f.blocks:
            insts = blk.instructions
            sem_incs: dict = {}  # (engine, sem_name) -> cumulative inc in stream
            new_insts = []
            for inst in insts:
                si = inst.sync_info
                if si is not None and si.on_wait:
                    cap = 2 if isinstance(inst, mybir.InstEventSemaphore) else 1
                    eng = inst.engine
                    pfx = _ENGINE_SEM_PREFIX.get(eng)
                    kept = []
                    for w in si.on_wait:
                        sem_eng = w.ant_name.rsplit("_", 1)[0]
                        if (
                            pfx is not None
                            and sem_eng == pfx
                            and w.wait_mode == "sem-ge-imm"
                            and sem_incs.get((eng, w.ant_name), 0) >= w.wait_value
                        ):
                            continue  # satisfied by same-engine execution order
                        kept.append(w)
                    while len(kept) > cap:
                        ncarry = min(2, len(kept) - cap + 1)
                        carry, kept = kept[:ncarry], kept[ncarry:]
                        ev = mybir.InstEventSemaphore(
                            name=f"EVW-{ev_id}", ins=[], outs=[]
                        )
                        ev_id += 1
                        ev.engine = eng
                        ev.sync_info = mybir.SyncInfo(on_wait=carry, on_update=[])
                        new_insts.append(ev)
                    inst.sync_info = mybir.SyncInfo(
                        on_wait=kept, on_update=si.on_update
                    )
                si = inst.sync_info
                if si is not None:
                    for u in si.on_update:
                        if u.update_mode == "sem-inc":
                            key = (inst.engine, u.ant_name)
                            sem_incs[key] = sem_incs.get(key, 0) + u.update_value
                new_insts.append(inst)
            if len(new_insts) != len(insts):
                insts[:] = new_insts


# Per-rank stage-1 mask engine split (32 tiles): DVE carries most, Pool and
# ACT absorb the rest so DVE can also build the stage-2 masks in time.
_S1_POOL_T = {2, 6, 11, 16, 21, 26, 30}  # 7 per rank on Pool
_S1_ACT_T = {4, 14, 24}  # 3 per rank on ACT

# Stage-2 chunk-copy engine rotation (PSUM->SBUF interleave): per 8 chunks.
_COPY_ROT = ["act", "dve", "pool", "act", "dve", "act", "dve", "pool"]


def _build_nc():
    nc = bass.Bass("TRN2", debug=False, num_devices=NCORES)

    # ft_in layout: [tables header (TAB fp16 cols) | ft columns (N)]
    #   hdr[0:G] = iota_g fp16; hdr[G:G+P] = identity fp16;
    #   hdr[G+P : G+P+644] = f32 table bitcast as fp16 pairs:
    #       f32[0:2] iota_p, f32[2:194] labT, f32[194:322] identity f32
    ft_in = nc.dram_tensor("ft_in", [P, TAB + N], FP16, kind="ExternalInput").ap()
    fb_in = nc.dram_tensor("fb_in", [P, N], FP16, kind="ExternalInput").ap()
    lab4_in = nc.dram_tensor("lab4_in", [R, N], FP16, kind="ExternalInput").ap()
    out = nc.dram_tensor("out", [BL, N, R + 1], F32, kind="ExternalOutput").ap()
    # tiny sink for the PE-warmup results (BIR verifier needs every location read)
    wsink = nc.dram_tensor("wsink", [P, 8], F32, kind="ExternalOutput").ap()

    with ExitStack() as ctx:
        tc = ctx.enter_context(tile.TileContext(nc))

        const = ctx.enter_context(tc.tile_pool(name="const", bufs=1))
        fpool = ctx.enter_context(tc.tile_pool(name="fpool", bufs=1))
        lbc = ctx.enter_context(tc.tile_pool(name="lbc", bufs=6))
        m1p = ctx.enter_context(tc.tile_pool(name="m1p", bufs=32))
        mabsp = ctx.enter_context(tc.tile_pool(name="mabsp", bufs=6))
        m2p = ctx.enter_context(tc.tile_pool(name="m2p", bufs=18))
        segp = ctx.enter_context(tc.tile_pool(name="segp", bufs=1))
        outp = ctx.enter_context(tc.tile_pool(name="outp", bufs=4))

        # ---- constant tables first (tiny), then F streams ----
        from concourse.tile import add_dep_helper

        # one combined header+ft tensor: tables arrive with the very first
        # DMA; lab4 goes last (not needed until mid-stage-1)
        hdr = fpool.tile([P, TAB + N], FP16)
        HCUTS = [0, TAB + 512, TAB + 2048, TAB + N]
        d_ft = []
        for ci in range(len(HCUTS) - 1):
            a, b = HCUTS[ci], HCUTS[ci + 1]
            d_ft.append(nc.sync.dma_start(hdr[:, a:b], ft_in[:, a:b]))
        lab4 = const.tile([P, N], FP16)
        d_lab = [
            nc.sync.dma_start(lab4[32 * a : 32 * a + R, :], lab4_in)
            for a in range(4)
        ]
        fb = fpool.tile([P, N], FP16)
        d_fb = nc.sync.dma_start(fb[:], fb_in)
        add_dep_helper(d_fb.ins, d_ft[-1].ins, reason="fb last")

        ft = hdr[:, TAB : TAB + N]
        f32v = hdr[:, G + P : TAB].bitcast(F32)  # [P, 322] f32 view
        iota_p = f32v[:, 0:2]
        labT = f32v[:, 2 : 2 + R * NT]
        ident_f = f32v[:, 2 + R * NT : 322]
        iota_g = hdr[:, 0:G]
        ident_h = hdr[:, G : G + P]

        # Prewarm: absorb const-DMA semaphores into engine clocks with cheap
        # ops so hot-loop instructions carry at most one sync wait each.
        wsrc = const.tile([P, P], FP16)
        nc.vector.memset(wsrc[:], 0)
        warm = const.tile([P, 8], F32)
        nc.vector.memset(warm[:], 0)
        # consume the preamble const tiles (BIR verifier wants readers)
        for _d, _v in (
            (mybir.dt.float32, 0.0),
            (mybir.dt.float32, 1.0),
            (mybir.dt.bfloat16, 1.0),
            (mybir.dt.uint8, 127),
        ):
            nc.vector.tensor_copy(warm[:, 4:5], nc.const_aps.aps[(_d, _v)])

        st = []  # seg_T fp16 per rank: st[r][g_loc, 128*gt + b]
        m2 = {}  # (r, gt, pi) -> [P, 1024] fp16 tile (first w cols valid)
        lab_bc = {}  # (r, pi) -> [P, 1024] fp16 broadcast label row

        # j-space piecing for lab_bc/m2: a small piece 0 (cheap to build on
        # DVE before the first store chunk), then 1024-wide pieces built
        # during the store phase.
        PIECES = [256] * 4 + [512] * 6  # sums to N
        POFF = [0]
        for w in PIECES:
            POFF.append(POFF[-1] + w)
        # store chunk widths (graduated: small first for early store start)
        CHUNKS = [64, 64, 128] + [256] * 14 + [128, 64, 64]  # sums to N
        COFF = [0]
        for w in CHUNKS:
            COFF.append(COFF[-1] + w)

        def piece_of(j0, j1):
            for pi in range(len(PIECES)):
                if POFF[pi] <= j0 and j1 <= POFF[pi + 1]:
                    return pi
            raise AssertionError((j0, j1))

        def shuffle_op(r, pi):
            w = PIECES[pi]
            t_l = lbc.tile([P, 1024], FP16, tag=f"lbc{pi % 2}", name=f"lbc{r}_{pi}")
            nc.vector.stream_shuffle(
                t_l[:, 0:w].bitcast(U32),
                lab4[:, POFF[pi] : POFF[pi] + w].bitcast(U32),
                [r] * 32,
            )
            lab_bc[(r, pi)] = t_l

        def m2_op(r, gt, pi):
            w = PIECES[pi]
            t_m = m2p.tile([P, 1024], FP16, tag=f"m2_{pi % 2}", name=f"m2_{r}_{gt}_{pi}")
            nc.vector.tensor_scalar(
                t_m[:, 0:w],
                lab_bc[(r, pi)][:, 0:w],
                iota_p[:, gt : gt + 1],
                None,
                op0=mybir.AluOpType.is_equal,
            )
            m2[(r, gt, pi)] = t_m

        with tc.tile_pool(name="ps_seg", bufs=1, space="PSUM") as ps_seg, \
                tc.tile_pool(name="ps_tr", bufs=2, space="PSUM") as ps_tr:
            # PE p-state warmup: small dependency-free matmuls start the ramp
            # clock before the real stage-1 work arrives.
            wps = [
                ps_tr.tile([P, G], F32, tag="tr", name=f"w{i}") for i in range(2)
            ]
            for i in range(28):
                nc.tensor.matmul(
                    wps[i % 2][:, 0:64],
                    wsrc[:],
                    wsrc[:, 0:64],
                    start=True,
                    stop=True,
                    skip_group_check=True,
                )
            # absorb the warm chain (verifier needs readers)
            nc.scalar.copy(warm[:, 6:7], wps[0][:, 0:1])
            nc.scalar.copy(warm[:, 7:8], wps[1][:, 0:1])
            nc.scalar.dma_start(wsink, warm[:])

            seg_tiles = [
                ps_seg.tile([P, G], F32, tag=f"segp{r}", name=f"segp{r}")
                for r in range(R)
            ]
            seg_ps = [seg_tiles[r][:] for r in range(R)]

            def s1_mask(r, t):
                col = r * NT + t
                idx = t * R + r
                m1 = m1p.tile([P, G], FP16, tag="m1")
                if idx % 10 == 3:  # ACT via abs+relu (exact for ints)
                    tmp = mabsp.tile([P, G], F32, tag="mabs")
                    nc.scalar.activation(
                        tmp[:],
                        iota_g,
                        mybir.ActivationFunctionType.Abs,
                        bias=labT[:, col : col + 1],
                        scale=-1.0,
                    )
                    nc.scalar.activation(
                        m1[:],
                        tmp[:],
                        mybir.ActivationFunctionType.Relu,
                        bias=1.0,
                        scale=-1.0,
                    )
                elif idx % 9 in (1, 5):  # Pool
                    nc.gpsimd.tensor_scalar(
                        m1[:],
                        iota_g,
                        labT[:, col : col + 1],
                        None,
                        op0=mybir.AluOpType.is_equal,
                    )
                else:  # DVE (4x fp16 mode)
                    nc.vector.tensor_scalar(
                        m1[:],
                        iota_g,
                        labT[:, col : col + 1],
                        None,
                        op0=mybir.AluOpType.is_equal,
                    )
                return m1

            def s1_matmul(r, t, m1):
                nc.tensor.matmul(
                    seg_ps[r],
                    ft[:, ts(t, P)],
                    m1[:],
                    start=(t == 0),
                    stop=(t == NT - 1),
                )

            # DVE inserts during stage 1: piece-0 shuffles + m2 (cheap) and
            # piece-1 shuffles, spread across the tile loop.
            ins_at = {}

            # tile-major over t=0..27 (tile t only needs its own ft quarter,
            # so PE never stalls on the F load), then per-rank finish for
            # t=28..31 so each rank's seg->seg_T chain overlaps the next
            # rank's matmuls.
            for t in range(NT - 4):
                for r in range(R):
                    s1_matmul(r, t, s1_mask(r, t))
                for op in ins_at.get(t, ()):
                    if op[0] == "sh":
                        shuffle_op(op[1], op[2])
                    else:
                        m2_op(op[1], op[2], op[3])
            # finish all remaining matmuls rank-by-rank, chains as each
            # rank completes; the copies ride the idle Pool engine so ACT is
            # free for the first chunk copies.
            pts = []
            for r in range(R):
                for t in range(NT - 4, NT):
                    s1_matmul(r, t, s1_mask(r, t))
                s_sb = segp.tile([P, G], F32, tag=f"ssb{r}", name=f"ssb{r}")
                if r % 2:
                    nc.vector.tensor_copy(s_sb[:], seg_ps[r])
                else:
                    nc.scalar.copy(s_sb[:], seg_ps[r])
                pt = ps_tr.tile([P, G], F32, tag="tr")
                for gt in range(2):
                    nc.tensor.transpose(
                        pt[:, ts(gt, P)], s_sb[:, ts(gt, P)], ident_f
                    )
                pts.append(pt)
                st_r = segp.tile([P, G], FP16, tag=f"st{r}", name=f"st{r}")
                nc.scalar.copy(st_r[:], pt[:])
                st.append(st_r)

        # ---- stage 2: per chunk, channel-major PSUM, one interleave copy
        # (ACT/Pool alternating), store on the SP queue. DVE runs the
        # remaining piece builds as one serial stream; it stays ahead of the
        # 2.55us/chunk store rate. ----
        dve_rest = []
        for pi in range(2):
            dve_rest += [("sh", r, pi) for r in range(R)]
            dve_rest += [("m2", r, gt, pi) for r in range(R) for gt in range(2)]
        for pi in range(2, len(PIECES)):
            dve_rest += [("sh", r, pi) for r in range(R)]
            dve_rest += [("m2", r, gt, pi) for r in range(R) for gt in range(2)]

        with tc.tile_pool(name="ps_o", bufs=2, space="PSUM") as ps_o:
            for op in dve_rest:
                if op[0] == "sh":
                    shuffle_op(op[1], op[2])
                else:
                    m2_op(op[1], op[2], op[3])
            for c, w in enumerate(CHUNKS):
                j0 = COFF[c]
                pi = piece_of(j0, j0 + w)
                off = j0 - POFF[pi]
                o_ps = ps_o.tile([P, R + 1, 256], F32, tag="o", name=f"o{c}")
                for r in range(R):
                    nc.tensor.matmul(
                        o_ps[:, r, 0:w],
                        st[r][:, 0:P],
                        m2[(r, 0, pi)][:, off : off + w],
                        start=True,
                        stop=False,
                    )
                    nc.tensor.matmul(
                        o_ps[:, r, 0:w],
                        st[r][:, P:G],
                        m2[(r, 1, pi)][:, off : off + w],
                        start=False,
                        stop=True,
                    )
                nc.tensor.matmul(
                    o_ps[:, R, 0:w],
                    ident_h,
                    fb[:, j0 : j0 + w],
                    start=True,
                    stop=True,
                )
                o_sb = outp.tile([P, 256, R + 1], F32, tag="ob", name=f"ob{c}")
                o_view = o_sb[:, 0:w, :].rearrange("p j c -> p c j")
                nc.scalar.copy(o_view, o_ps[:, :, 0:w])
                nc.sync.dma_start(out[:, j0 : j0 + w, :], o_sb[:, 0:w, :])

    _legalize_waits(nc)
    return nc


def _host_tables():
    """Packed fp16 header columns shared by all cores (cached)."""
    if "hdr" not in _cache:
        iota_g = np.tile(np.arange(G, dtype=np.float64), (P, 1)).astype(np.float16)
        ident_h = np.eye(P, dtype=np.float16)
        _cache["hdr"] = (iota_g, ident_h)
    return _cache["hdr"]


def kernel(F_genus: np.ndarray, labels: np.ndarray) -> np.ndarray:
    F_genus = np.ascontiguousarray(F_genus, dtype=np.float32)
    labels = np.ascontiguousarray(labels, dtype=np.int32)
    assert F_genus.shape == (B, N) and labels.shape == (R, N)

    iota_g, ident_h = _host_tables()
    # labT[p, r*NT + t] = labels[r, t*128 + p]
    labT = np.transpose(labels.reshape(R, NT, P), (2, 0, 1)).reshape(P, R * NT)
    iota_p = np.arange(P, dtype=np.float64)[:, None] + 128.0 * np.arange(2)[None, :]
    f32blk = np.concatenate(
        [iota_p, labT, np.eye(P)], axis=1
    ).astype(np.float32)  # [P, 322]
    f32_as_h = np.ascontiguousarray(f32blk).view(np.float16)  # [P, 644]
    hdr_cols = np.concatenate([iota_g, ident_h, f32_as_h], axis=1)  # [P, TAB]
    assert hdr_cols.shape == (P, TAB)
    lab4 = labels.astype(np.float16)

    in_maps = []
    for c in range(NCORES):
        Fc = F_genus[c * BL : (c + 1) * BL]
        # ft[i, t*128 + b] = Fc[b, t*128 + i]
        ft = np.ascontiguousarray(
            Fc.reshape(BL, NT, P).transpose(2, 1, 0).reshape(P, N)
        ).astype(np.float16)
        fbh = Fc.astype(np.float16)
        in_maps.append(
            {
                "ft_in": np.concatenate([hdr_cols, ft], axis=1),
                "fb_in": fbh,
                "lab4_in": lab4,
            }
        )

    # The first execution of a freshly compiled NEFF occasionally hits a
    # transient NRT_EXEC_UNIT_UNRECOVERABLE; a rebuild + retry recovers.
    last_err = None
    for attempt in range(3):
        try:
            if "nc" not in _cache:
                _cache["nc"] = _build_nc()
            res = run_bass_kernel_spmd(
                _cache["nc"], in_maps, core_ids=list(range(NCORES))
            )
            return np.concatenate([r["out"] for r in res.results], axis=0)
        except Exception as e:  # noqa: BLE001
            last_err = e
            _cache.pop("nc", None)
            import time as _time

            _time.sleep(3.0)
    raise last_err
